# revision 1
# baseline (speedup 1.0000x reference)
"""Self-contained Trainium2 Bass kernel for the 2-layer GAT problem.

Accepts FULL inputs, shards destination-node ranges across 8 NeuronCores
internally, and returns the FULL [50000, 2] float32 output.
"""
import numpy as np

import concourse.bacc as bacc
import concourse.mybir as mybir
import concourse.tile as tile
from concourse.masks import make_identity

F32 = mybir.dt.float32
BF = mybir.dt.bfloat16
I16 = mybir.dt.int16
NP_BF = mybir.dt.np(BF)

H = 8       # heads
C = 32      # per-head channels
HD = H * C  # 256
FIN = 128
ELEM = 384
ELEM2 = 128
P = 128

FULL_CFG = dict(
    N=50000, NPAD=50176, PER=6272, NBLK=49, HALF=25088, NSUB=10, NCORES=8,
    XCHUNK=1024,
)


def build_nc(cfg):
    NPAD, PER, NBLK, HALF, NSUB = (
        cfg["NPAD"], cfg["PER"], cfg["NBLK"], cfg["HALF"], cfg["NSUB"])
    NCORES = cfg["NCORES"]
    XCHUNK = cfg["XCHUNK"]
    NEH = NSUB * P                # idxs per (block, half) gather
    IDXW = NEH // 16              # idx cols per bucket
    NTILE = NPAD // P             # node tiles in P1
    assert NPAD == NCORES * PER and PER == NBLK * P and NPAD % XCHUNK == 0
    assert HALF % P == 0 and 2 * HALF == NPAD

    nc = bacc.Bacc(None, target_bir_lowering=False, num_devices=NCORES)

    xT_d = nc.dram_tensor("xT", [FIN, NPAD], F32, kind="ExternalInput")
    w1e_d = nc.dram_tensor("w1e", [FIN, 272], F32, kind="ExternalInput")
    w2e_d = nc.dram_tensor("w2e", [P, 8], BF, kind="ExternalInput")
    b1_d = nc.dram_tensor("b1r", [1, HD], F32, kind="ExternalInput")
    b2_d = nc.dram_tensor("b2r", [1, 2], F32, kind="ExternalInput")
    idx_d = nc.dram_tensor("idx16", [P, NBLK * 2 * IDXW], I16, kind="ExternalInput")
    dst_d = nc.dram_tensor("dstf", [P, NBLK * 2 * NSUB], F32, kind="ExternalInput")
    out_d = nc.dram_tensor("out", [PER, 2], F32, kind="ExternalOutput")

    table = nc.dram_tensor("table", [NPAD, ELEM], BF)
    t2loc = nc.dram_tensor("t2loc", [PER, ELEM2], BF)
    table2 = nc.dram_tensor("table2", [NPAD, ELEM2], BF)

    with tile.TileContext(nc) as tc:
        with (
            tc.tile_pool(name="cst", bufs=1) as cst,
            tc.tile_pool(name="xp", bufs=2) as xp,
            tc.tile_pool(name="rowp", bufs=3) as rowp,
            tc.tile_pool(name="gp", bufs=2) as gp,
            tc.tile_pool(name="g2p", bufs=2) as g2p,
            tc.tile_pool(name="wk", bufs=3) as wk,
            tc.tile_pool(name="tailp", bufs=2) as tailp,
            tc.tile_pool(name="ps", bufs=2, space="PSUM") as ps,
        ):
            # ---- constants ----
            ident = cst.tile([P, P], BF)
            make_identity(nc, ident[:])
            iota_i = cst.tile([P, P], I16)
            nc.gpsimd.iota(iota_i[:], pattern=[[1, P]], base=0, channel_multiplier=0)
            iota_bf = cst.tile([P, P], BF)
            nc.vector.tensor_copy(iota_bf[:], iota_i[:])
            onesk = cst.tile([1, P], F32)
            nc.vector.memset(onesk[:], 1.0)

            w1e_sb = cst.tile([FIN, 272], F32)
            nc.sync.dma_start(out=w1e_sb[:], in_=w1e_d[:])
            w2e_sb = cst.tile([P, 2, 4], BF)
            nc.sync.dma_start(out=w2e_sb[:], in_=w2e_d[:].rearrange("p (k n) -> p k n", k=2))
            idx_sb = cst.tile([P, NBLK * 2 * IDXW], I16)
            nc.sync.dma_start(out=idx_sb[:], in_=idx_d[:])
            dst_sb = cst.tile([P, NBLK * 2 * NSUB], F32)
            nc.sync.dma_start(out=dst_sb[:], in_=dst_d[:])

            # bias broadcast rows -> [P, HD], [P, 2]
            b1r = cst.tile([1, HD], F32)
            nc.sync.dma_start(out=b1r[:], in_=b1_d[:])
            b2r = cst.tile([1, 2], F32)
            nc.sync.dma_start(out=b2r[:], in_=b2_d[:])
            bps = ps.tile([P, HD], F32, space="PSUM", tag="accum")
            nc.tensor.matmul(out=bps[:], lhsT=onesk[:], rhs=b1r[:], start=True, stop=True)
            b1bc = cst.tile([P, HD], F32)
            nc.scalar.copy(b1bc[:], bps[:])
            bps2 = ps.tile([P, 2], F32, space="PSUM", tag="accum")
            nc.tensor.matmul(out=bps2[:], lhsT=onesk[:], rhs=b2r[:], start=True, stop=True)
            b2bc = cst.tile([P, 2], F32)
            nc.scalar.copy(b2bc[:], bps2[:])

            # ---- P1: node features -> table (replicated over all nodes) ----
            for ch in range(NPAD // XCHUNK):
                xc = xp.tile([FIN, XCHUNK], F32, tag="xc")
                nc.sync.dma_start(out=xc[:], in_=xT_d[:, ch * XCHUNK:(ch + 1) * XCHUNK])
                for j in range(XCHUNK // P):
                    nt = ch * (XCHUNK // P) + j
                    ph = ps.tile([P, 272], F32, space="PSUM", tag="accum")
                    nc.tensor.matmul(out=ph[:], lhsT=xc[:, j * P:(j + 1) * P],
                                     rhs=w1e_sb[:], start=True, stop=True)
                    row = rowp.tile([P, 280], BF, tag="row")
                    r3 = row[:, 0:264].rearrange("p (h x) -> p h x", h=H)
                    nc.scalar.copy(r3[:, :, 0:C],
                                   ph[:, 0:HD].rearrange("p (h c) -> p h c", h=H))
                    nc.gpsimd.memset(r3[:, :, C:C + 1], 1.0)
                    nc.scalar.copy(row[:, 264:280], ph[:, HD:HD + 16])
                    nc.sync.dma_start(out=table[nt * P:(nt + 1) * P, 0:280], in_=row[:])

            # ---- adst slice for own dst range (pid ladder) ----
            adst_sb = cst.tile([P, NBLK, H], BF)
            pid = nc.sync.partition_id()
            for c in range(NCORES):
                with tc.If(pid == c):
                    nc.sync.dma_start(
                        out=adst_sb[:],
                        in_=table[c * PER:(c + 1) * PER, 272:280]
                            .rearrange("(b p) h -> p b h", p=P))

            adst2_sb = cst.tile([P, NBLK], BF)

            # ---- P2: layer-1 message passing over own dst blocks ----
            for b in range(NBLK):
                pblk = ps.tile([P, 264], F32, space="PSUM", tag="accum")
                for half in (0, 1):
                    bucket = b * 2 + half
                    g = gp.tile([P, NSUB, ELEM], BF, tag="g")
                    nc.gpsimd.dma_gather(
                        out_ap=g[:],
                        in_ap=(table[0:HALF, :] if half == 0 else table[HALF:NPAD, :]),
                        idxs_ap=idx_sb[:, bucket * IDXW:(bucket + 1) * IDXW],
                        num_idxs=NEH, num_idxs_reg=NEH, elem_size=ELEM,
                        single_packet=False)
                    Ss = []
                    aexp = ps.tile([P, NSUB, H], F32, space="PSUM", tag="aexp")
                    for t in range(NSUB):
                        col = bucket * NSUB + t
                        S = wk.tile([P, P], BF, tag=f"S{t}", bufs=2)
                        nc.vector.tensor_scalar(
                            out=S[:], in0=iota_bf[:], scalar1=dst_sb[:, col:col + 1],
                            scalar2=None, op0=mybir.AluOpType.is_equal)
                        Ss.append(S)
                        T_ps = ps.tile([P, P], BF, space="PSUM", tag="tps")
                        nc.tensor.transpose(T_ps[:], S[:], ident[:])
                        T_sb = wk.tile([P, P], BF, tag="T_sb")
                        nc.scalar.copy(T_sb[:], T_ps[:])
                        nc.tensor.matmul(out=aexp[:, t, :], lhsT=T_sb[:],
                                         rhs=adst_sb[:, b, :], start=True, stop=True)
                    logits = wk.tile([P, NSUB, H], F32, tag="logits")
                    nc.vector.tensor_tensor(out=logits[:], in0=g[:, :, 264:272],
                                            in1=aexp[:], op=mybir.AluOpType.add)
                    e1 = wk.tile([P, NSUB, H], F32, tag="e1")
                    nc.scalar.activation(e1[:], logits[:], mybir.ActivationFunctionType.Exp)
                    e2 = wk.tile([P, NSUB, H], F32, tag="e2")
                    nc.scalar.activation(e2[:], logits[:], mybir.ActivationFunctionType.Exp,
                                         scale=0.2)
                    wt = wk.tile([P, NSUB, H], BF, tag="wt")
                    nc.vector.tensor_tensor(out=wt[:], in0=e1[:], in1=e2[:],
                                            op=mybir.AluOpType.max)
                    for t in range(NSUB):
                        msg = wk.tile([P, 264], BF, tag=f"msg{t % 3}")
                        nc.vector.tensor_tensor(
                            out=msg[:].rearrange("p (h x) -> p h x", h=H),
                            in0=g[:, t, 0:264].rearrange("p (h x) -> p h x", h=H),
                            in1=wt[:, t, :, None].to_broadcast([P, H, C + 1]),
                            op=mybir.AluOpType.mult)
                        nc.tensor.matmul(out=pblk[:], lhsT=Ss[t][:], rhs=msg[:],
                                         start=(half == 0 and t == 0),
                                         stop=(half == 1 and t == NSUB - 1))
                # tail: normalize + bias + ELU -> h2 -> t2loc rows
                pb3 = pblk[:].rearrange("p (h x) -> p h x", h=H)
                srec = tailp.tile([P, H], F32, tag="srec")
                nc.vector.tensor_scalar(
                    out=srec[:], in0=pb3[:, :, C:C + 1].rearrange("p h x -> p (h x)"),
                    scalar1=1e-16, scalar2=None, op0=mybir.AluOpType.add)
                rec = tailp.tile([P, H], F32, tag="rec")
                nc.vector.reciprocal(rec[:], srec[:])
                out1 = tailp.tile([P, HD], F32, tag="out1")
                nc.vector.tensor_tensor(
                    out=out1[:].rearrange("p (h c) -> p h c", h=H),
                    in0=pb3[:, :, 0:C],
                    in1=rec[:, :, None].to_broadcast([P, H, C]),
                    op=mybir.AluOpType.mult)
                v = tailp.tile([P, HD], F32, tag="v")
                nc.vector.tensor_tensor(out=v[:], in0=out1[:], in1=b1bc[:],
                                        op=mybir.AluOpType.add)
                ev = tailp.tile([P, HD], F32, tag="ev")
                nc.scalar.activation(ev[:], v[:], mybir.ActivationFunctionType.Exp)
                em = tailp.tile([P, HD], F32, tag="em")
                nc.vector.tensor_scalar(out=em[:], in0=ev[:], scalar1=1.0, scalar2=0.0,
                                        op0=mybir.AluOpType.subtract,
                                        op1=mybir.AluOpType.min)
                pp = tailp.tile([P, HD], F32, tag="pp")
                nc.scalar.activation(pp[:], v[:], mybir.ActivationFunctionType.Relu)
                elu = tailp.tile([P, HD], BF, tag="elu")
                nc.vector.tensor_tensor(out=elu[:], in0=em[:], in1=pp[:],
                                        op=mybir.AluOpType.add)
                eT_sb = tailp.tile([P, 2, P], BF, tag="eT_sb")
                ph2 = ps.tile([P, 4], F32, space="PSUM", tag="tail")
                for k in range(2):
                    eT_ps = ps.tile([P, P], BF, space="PSUM", tag="tps")
                    nc.tensor.transpose(eT_ps[:], elu[:, k * P:(k + 1) * P], ident[:])
                    nc.scalar.copy(eT_sb[:, k, :], eT_ps[:])
                for k in range(2):
                    nc.tensor.matmul(out=ph2[:], lhsT=eT_sb[:, k, :], rhs=w2e_sb[:, k, :],
                                     start=(k == 0), stop=(k == 1))
                t2row = tailp.tile([P, ELEM2], BF, tag="t2row")
                nc.scalar.copy(t2row[:, 0:2], ph2[:, 0:2])
                nc.gpsimd.memset(t2row[:, 2:3], 1.0)
                nc.scalar.copy(t2row[:, 3:5], ph2[:, 2:4])
                nc.gpsimd.memset(t2row[:, 5:ELEM2], 0.0)
                nc.sync.dma_start(out=t2loc[b * P:(b + 1) * P, :], in_=t2row[:])
                nc.scalar.copy(adst2_sb[:, b:b + 1], ph2[:, 3:4])

            # ---- AllGather layer-2 node table ----
            nc.gpsimd.collective_compute(
                "AllGather", mybir.AluOpType.bypass,
                replica_groups=[list(range(NCORES))],
                ins=[t2loc[:]], outs=[table2[:]])

            # ---- P3: layer-2 message passing ----
            for b in range(NBLK):
                p2s = ps.tile([P, 3], F32, space="PSUM", tag="accum")
                for half in (0, 1):
                    bucket = b * 2 + half
                    g2 = g2p.tile([P, NSUB, ELEM2], BF, tag="g2")
                    nc.gpsimd.dma_gather(
                        out_ap=g2[:],
                        in_ap=(table2[0:HALF, :] if half == 0 else table2[HALF:NPAD, :]),
                        idxs_ap=idx_sb[:, bucket * IDXW:(bucket + 1) * IDXW],
                        num_idxs=NEH, num_idxs_reg=NEH, elem_size=ELEM2,
                        single_packet=False)
                    S2s = []
                    a2e = ps.tile([P, NSUB], F32, space="PSUM", tag="aexp")
                    for t in range(NSUB):
                        col = bucket * NSUB + t
                        S2 = wk.tile([P, P], BF, tag=f"S{t}", bufs=2)
                        nc.vector.tensor_scalar(
                            out=S2[:], in0=iota_bf[:], scalar1=dst_sb[:, col:col + 1],
                            scalar2=None, op0=mybir.AluOpType.is_equal)
                        S2s.append(S2)
                        T2_ps = ps.tile([P, P], BF, space="PSUM", tag="tps")
                        nc.tensor.transpose(T2_ps[:], S2[:], ident[:])
                        T2_sb = wk.tile([P, P], BF, tag="T_sb")
                        nc.vector.tensor_copy(T2_sb[:], T2_ps[:])
                        nc.tensor.matmul(out=a2e[:, t:t + 1], lhsT=T2_sb[:],
                                         rhs=adst2_sb[:, b:b + 1], start=True, stop=True)
                    lg2 = wk.tile([P, NSUB], F32, tag="logits2")
                    nc.vector.tensor_tensor(
                        out=lg2[:], in0=g2[:, :, 3:4].rearrange("p t x -> p (t x)"),
                        in1=a2e[:], op=mybir.AluOpType.add)
                    f1 = wk.tile([P, NSUB], F32, tag="f1")
                    nc.scalar.activation(f1[:], lg2[:], mybir.ActivationFunctionType.Exp)
                    f2 = wk.tile([P, NSUB], F32, tag="f2")
                    nc.scalar.activation(f2[:], lg2[:], mybir.ActivationFunctionType.Exp,
                                         scale=0.2)
                    w2t = wk.tile([P, NSUB], BF, tag="w2t")
                    nc.vector.tensor_tensor(out=w2t[:], in0=f1[:], in1=f2[:],
                                            op=mybir.AluOpType.max)
                    for t in range(NSUB):
                        msg2 = wk.tile([P, 3], BF, tag=f"msg2{t % 3}")
                        nc.vector.tensor_tensor(
                            out=msg2[:], in0=g2[:, t, 0:3],
                            in1=w2t[:, t:t + 1].to_broadcast([P, 3]),
                            op=mybir.AluOpType.mult)
                        nc.tensor.matmul(out=p2s[:], lhsT=S2s[t][:], rhs=msg2[:],
                                         start=(half == 0 and t == 0),
                                         stop=(half == 1 and t == NSUB - 1))
                s2r = tailp.tile([P, 1], F32, tag="s2r")
                nc.vector.tensor_scalar(out=s2r[:], in0=p2s[:, 2:3], scalar1=1e-16,
                                        scalar2=None, op0=mybir.AluOpType.add)
                rec2 = tailp.tile([P, 1], F32, tag="rec2")
                nc.vector.reciprocal(rec2[:], s2r[:])
                o2 = tailp.tile([P, 2], F32, tag="o2")
                nc.vector.tensor_tensor(out=o2[:], in0=p2s[:, 0:2],
                                        in1=rec2[:].to_broadcast([P, 2]),
                                        op=mybir.AluOpType.mult)
                o2b = tailp.tile([P, 2], F32, tag="o2b")
                nc.vector.tensor_tensor(out=o2b[:], in0=o2[:], in1=b2bc[:],
                                        op=mybir.AluOpType.add)
                nc.sync.dma_start(out=out_d[b * P:(b + 1) * P, :], in_=o2b[:])

    nc.compile()
    return nc


def host_prep(inputs, cfg):
    """Build per-core input maps from full inputs."""
    N, NPAD, PER, NBLK, HALF, NSUB, NCORES = (
        cfg["N"], cfg["NPAD"], cfg["PER"], cfg["NBLK"], cfg["HALF"],
        cfg["NSUB"], cfg["NCORES"])
    NEH = NSUB * P
    IDXW = NEH // 16

    x = np.asarray(inputs["x"], dtype=np.float32)
    ei = np.asarray(inputs["edge_index"], dtype=np.int64)
    W1 = np.asarray(inputs["W1"], dtype=np.float64)
    a1s = np.asarray(inputs["a1_src"], dtype=np.float64)
    a1d = np.asarray(inputs["a1_dst"], dtype=np.float64)
    b1 = np.asarray(inputs["b1"], dtype=np.float32)
    W2 = np.asarray(inputs["W2"], dtype=np.float64)
    a2s = np.asarray(inputs["a2_src"], dtype=np.float64)
    a2d = np.asarray(inputs["a2_dst"], dtype=np.float64)
    b2 = np.asarray(inputs["b2"], dtype=np.float32)

    xT = np.zeros((FIN, NPAD), dtype=np.float32)
    xT[:, :N] = x.T

    A1s = np.zeros((HD, H))
    A1d = np.zeros((HD, H))
    for hd in range(H):
        A1s[hd * C:(hd + 1) * C, hd] = a1s[hd]
        A1d[hd * C:(hd + 1) * C, hd] = a1d[hd]
    w1e = np.concatenate([W1, W1 @ A1s, W1 @ A1d], axis=1).astype(np.float32)  # [128,272]

    w2cols = np.concatenate([W2, W2 @ a2s[0][:, None], W2 @ a2d[0][:, None]],
                            axis=1)  # [HD, 4]
    w2e = w2cols.reshape(2, P, 4).transpose(1, 0, 2).reshape(P, 8).astype(NP_BF)

    loops = np.arange(N, dtype=np.int64)
    src = np.concatenate([ei[0], loops])
    dst = np.concatenate([ei[1], loops])

    in_maps = []
    for c in range(NCORES):
        lo_n, hi_n = c * PER, (c + 1) * PER
        m = (dst >= lo_n) & (dst < hi_n)
        s_c = src[m]
        d_c = dst[m] - lo_n
        blk = d_c >> 7
        dloc = d_c & 127
        halfsel = (s_c >= HALF).astype(np.int64)
        key = blk * 2 + halfsel
        order = np.argsort(key, kind="stable")
        key_s = key[order]
        cnt = np.bincount(key_s, minlength=NBLK * 2)
        assert cnt.max() <= NEH, f"bucket overflow: {cnt.max()} > {NEH}"
        starts = np.zeros(NBLK * 2, dtype=np.int64)
        starts[1:] = np.cumsum(cnt)[:-1]
        pos = np.arange(len(key_s)) - starts[key_s]
        slot = key_s * NEH + pos
        idxflat = np.zeros(NBLK * 2 * NEH, dtype=np.int16)
        dstflat = np.full(NBLK * 2 * NEH, -1.0, dtype=np.float32)
        sv = s_c[order] - halfsel[order] * HALF
        idxflat[slot] = sv.astype(np.int16)
        dstflat[slot] = dloc[order].astype(np.float32)

        idxw16 = (idxflat.reshape(NBLK * 2, NSUB * 8, 16)
                  .transpose(2, 0, 1).reshape(16, -1))
        idxw = np.tile(idxw16, (8, 1))  # replicated across the 8 Q7 cores
        dstw = (dstflat.reshape(NBLK * 2, NSUB, P).transpose(2, 0, 1)
                .reshape(P, NBLK * 2 * NSUB))

        in_maps.append({
            "xT": xT, "w1e": w1e, "w2e": w2e,
            "b1r": b1.reshape(1, HD).astype(np.float32),
            "b2r": b2.reshape(1, 2).astype(np.float32),
            "idx16": idxw, "dstf": np.ascontiguousarray(dstw),
        })
    return in_maps


_NC_CACHE = {}


def _get_nc():
    if "nc" not in _NC_CACHE:
        _NC_CACHE["nc"] = build_nc(FULL_CFG)
    return _NC_CACHE["nc"]


def kernel(**inputs):
    from concourse.bass_utils import run_bass_kernel_spmd

    nc = _get_nc()
    in_maps = host_prep(inputs, FULL_CFG)
    res = run_bass_kernel_spmd(nc, in_maps, core_ids=list(range(FULL_CFG["NCORES"])))
    out = np.concatenate([r["out"] for r in res.results])[:FULL_CFG["N"]]
    return np.ascontiguousarray(out.astype(np.float32))



# revision 10
# speedup vs baseline: 1.0222x; 1.0222x over previous
"""Self-contained Trainium2 Bass kernel for the 2-layer GAT problem.

Accepts FULL inputs, shards destination-node ranges across 8 NeuronCores
internally, and returns the FULL [50000, 2] float32 output.

Structure (per core):
  P1: replicated node transform x@[W1|W1@A1s|W1@A1d] -> DRAM table rows
      [h(256) | asrc(8) | adst(8)] bf16, padded to 384-col rows (768B gather
      elems). adst captured into SBUF on the fly.
  P2: per (dst-block, src-half) bucket of <=1280 edges: dma_gather source
      rows (skip -1 pads), batched one-hot S build, PE-transpose -> ST,
      per-subtile adst broadcast via tiny matmuls, leakyrelu-softmax weights
      via exp/exp(0.2x)/max, aggregation matmuls with the weight column
      appended to the rhs (denominator accumulates in PSUM cols 256:264).
      Software-pipelined with a 1-bucket lookahead.
  AllGather of the layer-2 node table in 7 chunks (overlapped under P2),
      chunk-major table2 layout so each chunk is a contiguous AG output.
  P3: same machinery on the 4-wide layer-2 rows (256B gather elems).
"""
import numpy as np

import concourse.bacc as bacc
import concourse.mybir as mybir
import concourse.tile as tile
from concourse.masks import make_identity

F32 = mybir.dt.float32
BF = mybir.dt.bfloat16
I16 = mybir.dt.int16
NP_BF = mybir.dt.np(BF)

H = 8       # heads
C = 32      # per-head channels
HD = H * C  # 256
FIN = 128
ELEM = 384   # table row elems (768B); cols 0:272 used
ELEM2 = 128  # table2 row elems (256B); cols 0:4 used
P = 128

import os as _os

FULL_CFG = dict(
    N=50000, NPAD=50176, PER=6272, NBLK=49, HALF=25088, NSUB=10, NCORES=8,
    XCHUNK=1024,
    NAG=int(_os.environ.get("GAT_NAG", "7")),
    NEG=int(_os.environ.get("GAT_NEG", "1")),
)


def build_nc(cfg):
    NPAD, PER, NBLK, HALF, NSUB = (
        cfg["NPAD"], cfg["PER"], cfg["NBLK"], cfg["HALF"], cfg["NSUB"])
    NCORES = cfg["NCORES"]
    XCHUNK = cfg["XCHUNK"]
    NAG = cfg["NAG"]
    NEH = NSUB * P                # idxs per (block, half) gather
    IDXW = NEH // 16              # idx cols per bucket
    NTILE = NPAD // P             # node tiles in P1
    NB2 = NBLK * 2                # buckets
    CBLK = NBLK // NAG            # blocks per AG chunk
    CROWS = CBLK * P              # local rows per AG chunk
    assert NPAD == NCORES * PER and PER == NBLK * P and NPAD % XCHUNK == 0
    assert HALF % P == 0 and 2 * HALF == NPAD and NBLK == NAG * CBLK

    nc = bacc.Bacc(None, target_bir_lowering=False, num_devices=NCORES)

    xT_d = nc.dram_tensor("xT", [FIN, NPAD], BF, kind="ExternalInput")
    w1e_d = nc.dram_tensor("w1e", [FIN, 272], BF, kind="ExternalInput")
    w2e_d = nc.dram_tensor("w2e", [P, 8], BF, kind="ExternalInput")
    b1_d = nc.dram_tensor("b1r", [1, HD], F32, kind="ExternalInput")
    b2_d = nc.dram_tensor("b2r", [1, 2], F32, kind="ExternalInput")
    idx_d = nc.dram_tensor("idx16", [P, NB2 * IDXW], I16, kind="ExternalInput")
    idx2_d = nc.dram_tensor("idx16b", [P, NB2 * IDXW], I16, kind="ExternalInput")
    dst_d = nc.dram_tensor("dstf", [P, NB2 * NSUB], BF, kind="ExternalInput")
    dst2_d = nc.dram_tensor("dstfb", [P, NB2 * NSUB], BF, kind="ExternalInput")
    out_d = nc.dram_tensor("out", [PER, 2], F32, kind="ExternalOutput")

    table = nc.dram_tensor("table", [NPAD, ELEM], BF)
    t2locs = [nc.dram_tensor(f"t2loc{k}", [CROWS, ELEM2], BF) for k in range(NAG)]
    table2 = nc.dram_tensor("table2", [NPAD, ELEM2], BF)

    with tile.TileContext(nc) as tc:
        with (
            tc.tile_pool(name="cst", bufs=1) as cst,
            tc.tile_pool(name="xp", bufs=2) as xp,
            tc.tile_pool(name="rowp", bufs=3) as rowp,
            tc.tile_pool(name="gp", bufs=2) as gp,
            tc.tile_pool(name="g2p", bufs=2) as g2p,
            tc.tile_pool(name="sp", bufs=3) as sp,
            tc.tile_pool(name="mp", bufs=2) as mp,
            tc.tile_pool(name="tailp", bufs=2) as tailp,
            tc.tile_pool(name="ps", bufs=2, space="PSUM") as ps,
        ):
            # ---- constants ----
            ident = cst.tile([P, P], BF)
            make_identity(nc, ident[:])
            iota_i = cst.tile([P, P], I16)
            nc.gpsimd.iota(iota_i[:], pattern=[[1, P]], base=0, channel_multiplier=0)
            iota_bf = cst.tile([P, P], BF)
            nc.vector.tensor_copy(iota_bf[:], iota_i[:])
            iota_rep = cst.tile([P, NSUB, P], BF)
            nc.vector.tensor_copy(
                iota_rep[:], iota_bf[:, None, :].to_broadcast([P, NSUB, P]))
            onesk = cst.tile([1, P], F32)
            nc.vector.memset(onesk[:], 1.0)

            w1e_sb = cst.tile([FIN, 272], BF)
            nc.sync.dma_start(out=w1e_sb[:], in_=w1e_d[:])
            w2e_sb = cst.tile([P, 2, 4], BF)
            nc.sync.dma_start(out=w2e_sb[:], in_=w2e_d[:].rearrange("p (k n) -> p k n", k=2))
            idx_sb = cst.tile([P, NB2 * IDXW], I16)
            nc.sync.dma_start(out=idx_sb[:], in_=idx_d[:])
            idx2_sb = cst.tile([P, NB2 * IDXW], I16)
            nc.sync.dma_start(out=idx2_sb[:], in_=idx2_d[:])
            dst_sb = cst.tile([P, NB2 * NSUB], BF)
            nc.sync.dma_start(out=dst_sb[:], in_=dst_d[:])
            dst2_sb = cst.tile([P, NB2 * NSUB], BF)
            nc.sync.dma_start(out=dst2_sb[:], in_=dst2_d[:])

            # bias broadcast rows -> [P, HD], [P, 2]
            b1r = cst.tile([1, HD], F32)
            nc.sync.dma_start(out=b1r[:], in_=b1_d[:])
            b2r = cst.tile([1, 2], F32)
            nc.sync.dma_start(out=b2r[:], in_=b2_d[:])
            bps = ps.tile([P, HD], F32, space="PSUM", tag="aexp")
            nc.tensor.matmul(out=bps[:], lhsT=onesk[:], rhs=b1r[:], start=True, stop=True)
            b1bc = cst.tile([P, HD], F32)
            nc.scalar.copy(b1bc[:], bps[:])
            bps2 = ps.tile([P, 2], F32, space="PSUM", tag="aexp")
            nc.tensor.matmul(out=bps2[:], lhsT=onesk[:], rhs=b2r[:], start=True, stop=True)
            b2bc = cst.tile([P, 2], F32)
            nc.scalar.copy(b2bc[:], bps2[:])

            adst_sb = cst.tile([P, NBLK, H], BF)
            adst2_sb = cst.tile([P, NBLK], BF)
            outstage = cst.tile([P, NBLK, 2], F32)

            # prime gather buffers so skipped (-1) slots read finite data
            for _ in range(2):
                gz = gp.tile([P, NSUB, ELEM], BF, tag="g")
                nc.vector.memset(gz[:], 0.0)
                g2z = g2p.tile([P, NSUB, ELEM2], BF, tag="g2")
                nc.vector.memset(g2z[:], 0.0)

            # ---- P1: node features -> table (replicated over all nodes) ----
            for ch in range(NPAD // XCHUNK):
                xc = xp.tile([FIN, XCHUNK], BF, tag="xc")
                nc.sync.dma_start(out=xc[:], in_=xT_d[:, ch * XCHUNK:(ch + 1) * XCHUNK])
                for j in range(XCHUNK // P):
                    nt = ch * (XCHUNK // P) + j
                    ph = ps.tile([P, 272], F32, space="PSUM", tag="accum")
                    nc.tensor.matmul(out=ph[:], lhsT=xc[:, j * P:(j + 1) * P],
                                     rhs=w1e_sb[:], start=True, stop=True)
                    row = rowp.tile([P, 272], BF, tag="row")
                    if nt % 2 == 0:
                        nc.scalar.copy(row[:], ph[:])
                    else:
                        nc.vector.tensor_copy(row[:], ph[:])
                    nc.sync.dma_start(out=table[nt * P:(nt + 1) * P, 0:272], in_=row[:])

            # ---- adst slice for own dst range (pid ladder) ----
            pid = nc.sync.partition_id()
            for c in range(NCORES):
                with tc.If(pid == c):
                    nc.sync.dma_start(
                        out=adst_sb[:],
                        in_=table[c * PER:(c + 1) * PER, 264:272]
                            .rearrange("(b p) h -> p b h", p=P))

            # =========== P2: layer-1 message passing (pipelined) ===========
            state = {}
            pblks = {}

            def prep2(k):
                b, half = k // 2, k % 2
                g = gp.tile([P, NSUB, ELEM], BF, tag="g")
                nc.gpsimd.dma_gather(
                    out_ap=g[:],
                    in_ap=(table[0:HALF, :] if half == 0 else table[HALF:NPAD, :]),
                    idxs_ap=idx_sb[:, k * IDXW:(k + 1) * IDXW],
                    num_idxs=NEH, num_idxs_reg=NEH, elem_size=ELEM,
                    single_packet=False)
                S = sp.tile([P, NSUB, P], BF, tag="S")
                nc.vector.tensor_tensor(
                    out=S[:], in0=iota_rep[:],
                    in1=dst_sb[:, k * NSUB:(k + 1) * NSUB][:, :, None]
                        .to_broadcast([P, NSUB, P]),
                    op=mybir.AluOpType.is_equal)
                ST = sp.tile([P, NSUB, P], BF, tag="ST", bufs=2)
                for grp in range(2):
                    tps = ps.tile([P, 5, P], BF, space="PSUM", tag="tps")
                    for tt in range(5):
                        t = grp * 5 + tt
                        nc.tensor.transpose(tps[:, tt, :], S[:, t, :], ident[:])
                    nc.scalar.copy(ST[:, grp * 5:(grp + 1) * 5, :], tps[:])
                aexp = ps.tile([P, NSUB, H], F32, space="PSUM", tag="aexp")
                for t in range(NSUB):
                    nc.tensor.matmul(out=aexp[:, t, :], lhsT=ST[:, t, :],
                                     rhs=adst_sb[:, b, :], start=True, stop=True)
                logits = mp.tile([P, NSUB, H], F32, tag="logits")
                nc.vector.tensor_tensor(out=logits[:], in0=g[:, :, 256:264],
                                        in1=aexp[:], op=mybir.AluOpType.add)
                e1 = mp.tile([P, NSUB, H], F32, tag="e1")
                nc.scalar.activation(e1[:], logits[:], mybir.ActivationFunctionType.Exp)
                e2 = mp.tile([P, NSUB, H], F32, tag="e2")
                nc.scalar.activation(e2[:], logits[:], mybir.ActivationFunctionType.Exp,
                                     scale=0.2)
                wt = mp.tile([P, NSUB, H], F32, tag="wt")
                nc.vector.tensor_tensor(out=wt[:], in0=e1[:], in1=e2[:],
                                        op=mybir.AluOpType.max)
                msg = mp.tile([P, NSUB, 264], BF, tag="msg")
                nc.vector.tensor_tensor(
                    out=msg[:, :, 0:256].rearrange("p t (h c) -> p t h c", h=H),
                    in0=g[:, :, 0:256].rearrange("p t (h c) -> p t h c", h=H),
                    in1=wt[:, :, :, None].to_broadcast([P, NSUB, H, C]),
                    op=mybir.AluOpType.mult)
                nc.vector.tensor_copy(msg[:, :, 256:264], wt[:])
                state[k] = (S, msg)

            def agg2(k):
                b, half = k // 2, k % 2
                S, msg = state.pop(k)
                if half == 0:
                    pblks[b] = ps.tile([P, 264], F32, space="PSUM", tag="accum", name="pblk")
                pblk = pblks[b]
                for t in range(NSUB):
                    nc.tensor.matmul(out=pblk[:], lhsT=S[:, t, :], rhs=msg[:, t, :],
                                     start=(half == 0 and t == 0),
                                     stop=(half == 1 and t == NSUB - 1))

            def tail1(b):
                pb = pblks.pop(b)
                srec = tailp.tile([P, H], F32, tag="srec")
                nc.vector.tensor_scalar(
                    out=srec[:], in0=pb[:, 256:264], scalar1=1e-16, scalar2=None,
                    op0=mybir.AluOpType.add)
                rec = tailp.tile([P, H], F32, tag="rec")
                nc.vector.reciprocal(rec[:], srec[:])
                out1 = tailp.tile([P, HD], F32, tag="out1")
                nc.vector.tensor_tensor(
                    out=out1[:].rearrange("p (h c) -> p h c", h=H),
                    in0=pb[:, 0:256].rearrange("p (h c) -> p h c", h=H),
                    in1=rec[:, :, None].to_broadcast([P, H, C]),
                    op=mybir.AluOpType.mult)
                v = tailp.tile([P, HD], F32, tag="v")
                nc.vector.tensor_tensor(out=v[:], in0=out1[:], in1=b1bc[:],
                                        op=mybir.AluOpType.add)
                ev = tailp.tile([P, HD], F32, tag="ev")
                nc.scalar.activation(ev[:], v[:], mybir.ActivationFunctionType.Exp)
                em = tailp.tile([P, HD], F32, tag="em")
                nc.vector.tensor_scalar(out=em[:], in0=ev[:], scalar1=1.0, scalar2=0.0,
                                        op0=mybir.AluOpType.subtract,
                                        op1=mybir.AluOpType.min)
                pp = tailp.tile([P, HD], F32, tag="pp")
                nc.vector.tensor_scalar(out=pp[:], in0=v[:], scalar1=0.0, scalar2=None,
                                        op0=mybir.AluOpType.max)
                elu = tailp.tile([P, HD], BF, tag="elu")
                nc.vector.tensor_tensor(out=elu[:], in0=em[:], in1=pp[:],
                                        op=mybir.AluOpType.add)
                eT_ps = ps.tile([P, 2, P], BF, space="PSUM", tag="tps")
                for kk in range(2):
                    nc.tensor.transpose(eT_ps[:, kk, :], elu[:, kk * P:(kk + 1) * P],
                                        ident[:])
                eT_sb = tailp.tile([P, 2, P], BF, tag="eT")
                nc.scalar.copy(eT_sb[:], eT_ps[:])
                ph2 = ps.tile([P, 4], F32, space="PSUM", tag="aexp")
                for kk in range(2):
                    nc.tensor.matmul(out=ph2[:], lhsT=eT_sb[:, kk, :],
                                     rhs=w2e_sb[:, kk, :],
                                     start=(kk == 0), stop=(kk == 1))
                t2row = tailp.tile([P, 4], BF, tag="t2row")
                nc.vector.tensor_copy(t2row[:], ph2[:, 0:4])
                nc.vector.tensor_copy(adst2_sb[:, b:b + 1], ph2[:, 3:4])
                kag = b // CBLK
                rb = (b % CBLK) * P
                nc.sync.dma_start(out=t2locs[kag][rb:rb + P, 0:4], in_=t2row[:])

            for k in range(NB2 + 1):
                if k >= 1:
                    agg2(k - 1)
                    if (k - 1) % 2 == 1:
                        b = (k - 1) // 2
                        tail1(b)
                        if (b + 1) % CBLK == 0:
                            kag = b // CBLK
                            nc.gpsimd.collective_compute(
                                "AllGather", mybir.AluOpType.bypass,
                                replica_groups=[list(range(NCORES))],
                                ins=[t2locs[kag][:]],
                                outs=[table2[kag * NCORES * CROWS:
                                             (kag + 1) * NCORES * CROWS, :]])
                if k < NB2:
                    prep2(k)

            # =========== P3: layer-2 message passing (pipelined) ===========
            def prep3(k):
                b, half = k // 2, k % 2
                g2 = g2p.tile([P, NSUB, ELEM2], BF, tag="g2")
                nc.gpsimd.dma_gather(
                    out_ap=g2[:],
                    in_ap=(table2[0:HALF, :] if half == 0 else table2[HALF:NPAD, :]),
                    idxs_ap=idx2_sb[:, k * IDXW:(k + 1) * IDXW],
                    num_idxs=NEH, num_idxs_reg=NEH, elem_size=ELEM2,
                    single_packet=False)
                S = sp.tile([P, NSUB, P], BF, tag="S")
                nc.vector.tensor_tensor(
                    out=S[:], in0=iota_rep[:],
                    in1=dst2_sb[:, k * NSUB:(k + 1) * NSUB][:, :, None]
                        .to_broadcast([P, NSUB, P]),
                    op=mybir.AluOpType.is_equal)
                ST = sp.tile([P, NSUB, P], BF, tag="ST", bufs=2)
                for grp in range(2):
                    tps = ps.tile([P, 5, P], BF, space="PSUM", tag="tps")
                    for tt in range(5):
                        t = grp * 5 + tt
                        nc.tensor.transpose(tps[:, tt, :], S[:, t, :], ident[:])
                    nc.scalar.copy(ST[:, grp * 5:(grp + 1) * 5, :], tps[:])
                a2e = ps.tile([P, NSUB], F32, space="PSUM", tag="aexp")
                for t in range(NSUB):
                    nc.tensor.matmul(out=a2e[:, t:t + 1], lhsT=ST[:, t, :],
                                     rhs=adst2_sb[:, b:b + 1], start=True, stop=True)
                lg2 = mp.tile([P, NSUB], F32, tag="lg2")
                nc.vector.tensor_tensor(
                    out=lg2[:], in0=g2[:, :, 2:3].rearrange("p t x -> p (t x)"),
                    in1=a2e[:], op=mybir.AluOpType.add)
                f1 = mp.tile([P, NSUB], F32, tag="f1")
                nc.scalar.activation(f1[:], lg2[:], mybir.ActivationFunctionType.Exp)
                f2 = mp.tile([P, NSUB], F32, tag="f2")
                nc.scalar.activation(f2[:], lg2[:], mybir.ActivationFunctionType.Exp,
                                     scale=0.2)
                w2t = mp.tile([P, NSUB], F32, tag="w2t")
                nc.vector.tensor_tensor(out=w2t[:], in0=f1[:], in1=f2[:],
                                        op=mybir.AluOpType.max)
                msg2 = mp.tile([P, NSUB, 3], BF, tag="msg2")
                nc.vector.tensor_tensor(
                    out=msg2[:, :, 0:2], in0=g2[:, :, 0:2],
                    in1=w2t[:, :, None].to_broadcast([P, NSUB, 2]),
                    op=mybir.AluOpType.mult)
                nc.vector.tensor_copy(msg2[:, :, 2:3], w2t[:, :, None])
                state[k] = (S, msg2)

            def agg3(k):
                b, half = k // 2, k % 2
                S, msg2 = state.pop(k)
                if half == 0:
                    pblks[b] = ps.tile([P, 3], F32, space="PSUM", tag="accum", name="p2s")
                p2s = pblks[b]
                for t in range(NSUB):
                    nc.tensor.matmul(out=p2s[:], lhsT=S[:, t, :], rhs=msg2[:, t, :],
                                     start=(half == 0 and t == 0),
                                     stop=(half == 1 and t == NSUB - 1))

            def tail3(b):
                p2s = pblks.pop(b)
                s2r = tailp.tile([P, 1], F32, tag="s2r")
                nc.vector.tensor_scalar(out=s2r[:], in0=p2s[:, 2:3], scalar1=1e-16,
                                        scalar2=None, op0=mybir.AluOpType.add)
                rec2 = tailp.tile([P, 1], F32, tag="rec2")
                nc.vector.reciprocal(rec2[:], s2r[:])
                o2 = tailp.tile([P, 2], F32, tag="o2")
                nc.vector.tensor_tensor(out=o2[:], in0=p2s[:, 0:2],
                                        in1=rec2[:].to_broadcast([P, 2]),
                                        op=mybir.AluOpType.mult)
                nc.vector.tensor_tensor(out=outstage[:, b, :], in0=o2[:], in1=b2bc[:],
                                        op=mybir.AluOpType.add)

            for k in range(NB2 + 1):
                if k >= 1:
                    agg3(k - 1)
                    if (k - 1) % 2 == 1:
                        tail3((k - 1) // 2)
                if k < NB2:
                    prep3(k)

            nc.sync.dma_start(
                out=out_d[:].rearrange("(b p) c -> p b c", p=P), in_=outstage[:])

    nc.compile()
    return nc


def host_prep(inputs, cfg):
    """Build per-core input maps from full inputs."""
    N, NPAD, PER, NBLK, HALF, NSUB, NCORES, NAG = (
        cfg["N"], cfg["NPAD"], cfg["PER"], cfg["NBLK"], cfg["HALF"],
        cfg["NSUB"], cfg["NCORES"], cfg["NAG"])
    NEH = NSUB * P
    IDXW = NEH // 16
    CBLK = NBLK // NAG

    x = np.asarray(inputs["x"], dtype=np.float32)
    ei = np.asarray(inputs["edge_index"], dtype=np.int64)
    W1 = np.asarray(inputs["W1"], dtype=np.float64)
    a1s = np.asarray(inputs["a1_src"], dtype=np.float64)
    a1d = np.asarray(inputs["a1_dst"], dtype=np.float64)
    b1 = np.asarray(inputs["b1"], dtype=np.float32)
    W2 = np.asarray(inputs["W2"], dtype=np.float64)
    a2s = np.asarray(inputs["a2_src"], dtype=np.float64)
    a2d = np.asarray(inputs["a2_dst"], dtype=np.float64)
    b2 = np.asarray(inputs["b2"], dtype=np.float32)

    xT = np.zeros((FIN, NPAD), dtype=np.float32)
    xT[:, :N] = x.T
    xT = xT.astype(NP_BF)

    A1s = np.zeros((HD, H))
    A1d = np.zeros((HD, H))
    for hd in range(H):
        A1s[hd * C:(hd + 1) * C, hd] = a1s[hd]
        A1d[hd * C:(hd + 1) * C, hd] = a1d[hd]
    w1e = np.concatenate([W1, W1 @ A1s, W1 @ A1d], axis=1).astype(NP_BF)  # [128,272]

    w2cols = np.concatenate([W2, W2 @ a2s[0][:, None], W2 @ a2d[0][:, None]],
                            axis=1)  # [HD, 4]
    w2e = w2cols.reshape(2, P, 4).transpose(1, 0, 2).reshape(P, 8).astype(NP_BF)

    loops = np.arange(N, dtype=np.int64)
    src = np.concatenate([ei[0], loops])
    dst = np.concatenate([ei[1], loops])

    # chunk-major table2 row index for each node
    nodes = np.arange(NPAD, dtype=np.int64)
    n_c, n_r = nodes // PER, nodes % PER
    n_b, n_j = n_r >> 7, n_r & 127
    n_k = n_b // CBLK
    row2_of = (n_k * (NCORES * CBLK * P) + n_c * (CBLK * P)
               + (n_b - n_k * CBLK) * P + n_j)

    in_maps = []
    for c in range(NCORES):
        lo_n, hi_n = c * PER, (c + 1) * PER
        m = (dst >= lo_n) & (dst < hi_n)
        s_c = src[m]
        d_c = dst[m] - lo_n
        blk = d_c >> 7
        dloc = d_c & 127

        packs = []
        for srow in (s_c, row2_of[s_c]):
            halfsel = (srow >= HALF).astype(np.int64)
            key = blk * 2 + halfsel
            order = np.argsort(key, kind="stable")
            key_s = key[order]
            cnt = np.bincount(key_s, minlength=NBLK * 2)
            assert cnt.max() <= NEH, f"bucket overflow: {cnt.max()} > {NEH}"
            starts = np.zeros(NBLK * 2, dtype=np.int64)
            starts[1:] = np.cumsum(cnt)[:-1]
            pos = np.arange(len(key_s)) - starts[key_s]
            slot = key_s * NEH + pos
            pad_idx = -1 if cfg.get("NEG", 1) else 0
            idxflat = np.full(NBLK * 2 * NEH, pad_idx, dtype=np.int16)
            dstflat = np.full(NBLK * 2 * NEH, -1.0, dtype=np.float32)
            sv = srow[order] - halfsel[order] * HALF
            idxflat[slot] = sv.astype(np.int16)
            dstflat[slot] = dloc[order].astype(np.float32)
            idxw16 = (idxflat.reshape(NBLK * 2, NSUB * 8, 16)
                      .transpose(2, 0, 1).reshape(16, -1))
            idxw = np.tile(idxw16, (8, 1))  # replicated across the 8 Q7 cores
            dstw = (dstflat.reshape(NBLK * 2, NSUB, P).transpose(2, 0, 1)
                    .reshape(P, NBLK * 2 * NSUB)).astype(NP_BF)
            packs.append((idxw, np.ascontiguousarray(dstw)))

        in_maps.append({
            "xT": xT, "w1e": w1e, "w2e": w2e,
            "b1r": b1.reshape(1, HD).astype(np.float32),
            "b2r": b2.reshape(1, 2).astype(np.float32),
            "idx16": packs[0][0], "dstf": packs[0][1],
            "idx16b": packs[1][0], "dstfb": packs[1][1],
        })
    return in_maps


_NC_CACHE = {}


def _get_nc():
    if "nc" not in _NC_CACHE:
        _NC_CACHE["nc"] = build_nc(FULL_CFG)
    return _NC_CACHE["nc"]


def kernel(**inputs):
    from concourse.bass_utils import run_bass_kernel_spmd

    nc = _get_nc()
    in_maps = host_prep(inputs, FULL_CFG)
    res = run_bass_kernel_spmd(nc, in_maps, core_ids=list(range(FULL_CFG["NCORES"])))
    out = np.concatenate([r["out"] for r in res.results])[:FULL_CFG["N"]]
    return np.ascontiguousarray(out.astype(np.float32))


# revision 12
# speedup vs baseline: 1.1844x; 1.1587x over previous
"""Self-contained Trainium2 Bass kernel for the 2-layer GAT problem.

Accepts FULL inputs, shards destination-node ranges across 8 NeuronCores
internally, and returns the FULL [50000, 2] float32 output.

Structure (per core):
  P1: replicated node transform x@[W1|W1@A1s|W1@A1d] -> DRAM table rows
      [h(256) | asrc(8) | adst(8)] bf16, padded to 384-col rows (768B gather
      elems). adst captured into SBUF on the fly.
  P2: per (dst-block, src-half) bucket of <=1280 edges: dma_gather source
      rows (skip -1 pads), batched one-hot S build, PE-transpose -> ST,
      per-subtile adst broadcast via tiny matmuls, leakyrelu-softmax weights
      via exp/exp(0.2x)/max, aggregation matmuls with the weight column
      appended to the rhs (denominator accumulates in PSUM cols 256:264).
      Software-pipelined with a 1-bucket lookahead.
  AllGather of the layer-2 node table in 7 chunks (overlapped under P2),
      chunk-major table2 layout so each chunk is a contiguous AG output.
  P3: same machinery on the 4-wide layer-2 rows (256B gather elems).
"""
import numpy as np

import concourse.bacc as bacc
import concourse.mybir as mybir
import concourse.tile as tile
from concourse.masks import make_identity

F32 = mybir.dt.float32
BF = mybir.dt.bfloat16
I16 = mybir.dt.int16
NP_BF = mybir.dt.np(BF)

H = 8       # heads
C = 32      # per-head channels
HD = H * C  # 256
FIN = 128
ELEM = 384   # table row elems (768B); cols 0:272 used
ELEM2 = 128  # table2 row elems (256B); cols 0:4 used
P = 128

import os as _os

FULL_CFG = dict(
    N=50000, NPAD=50176, PER=6272, NBLK=49, HALF=25088, NSUB=10, NCORES=8,
    XCHUNK=1024,
    NAG=int(_os.environ.get("GAT_NAG", "7")),
    NEG=int(_os.environ.get("GAT_NEG", "1")),
    NSWQ=int(_os.environ.get("GAT_NSWQ", "1")),
    SP=int(_os.environ.get("GAT_SP", "0")),
)


def build_nc(cfg):
    NPAD, PER, NBLK, HALF, NSUB = (
        cfg["NPAD"], cfg["PER"], cfg["NBLK"], cfg["HALF"], cfg["NSUB"])
    NCORES = cfg["NCORES"]
    XCHUNK = cfg["XCHUNK"]
    NAG = cfg["NAG"]
    NEH = NSUB * P                # idxs per (block, half) gather
    IDXW = NEH // 16              # idx cols per bucket
    NTILE = NPAD // P             # node tiles in P1
    NB2 = NBLK * 2                # buckets
    CBLK = NBLK // NAG            # blocks per AG chunk
    CROWS = CBLK * P              # local rows per AG chunk
    assert NPAD == NCORES * PER and PER == NBLK * P and NPAD % XCHUNK == 0
    assert HALF % P == 0 and 2 * HALF == NPAD and NBLK == NAG * CBLK

    NSWQ = cfg.get("NSWQ", 1)
    SP = bool(cfg.get("SP", 0))
    nc = bacc.Bacc(None, target_bir_lowering=False, num_devices=NCORES,
                   num_swdge_queues=NSWQ)

    xT_d = nc.dram_tensor("xT", [FIN, NPAD], BF, kind="ExternalInput")
    w1e_d = nc.dram_tensor("w1e", [FIN, 272], BF, kind="ExternalInput")
    w2e_d = nc.dram_tensor("w2e", [P, 8], BF, kind="ExternalInput")
    b1_d = nc.dram_tensor("b1r", [1, HD], F32, kind="ExternalInput")
    b2_d = nc.dram_tensor("b2r", [1, 2], F32, kind="ExternalInput")
    idx_d = nc.dram_tensor("idx16", [P, NB2 * IDXW], I16, kind="ExternalInput")
    idx2_d = nc.dram_tensor("idx16b", [P, NB2 * IDXW], I16, kind="ExternalInput")
    dst_d = nc.dram_tensor("dstf", [P, NB2 * NSUB], BF, kind="ExternalInput")
    dst2_d = nc.dram_tensor("dstfb", [P, NB2 * NSUB], BF, kind="ExternalInput")
    out_d = nc.dram_tensor("out", [PER, 2], F32, kind="ExternalOutput")

    table = nc.dram_tensor("table", [NPAD, ELEM], BF)
    t2locs = [nc.dram_tensor(f"t2loc{k}", [CROWS, ELEM2], BF) for k in range(NAG)]
    table2 = nc.dram_tensor("table2", [NPAD, ELEM2], BF)

    with tile.TileContext(nc) as tc:
        with (
            tc.tile_pool(name="cst", bufs=1) as cst,
            tc.tile_pool(name="xp", bufs=2) as xp,
            tc.tile_pool(name="rowp", bufs=3) as rowp,
            tc.tile_pool(name="gp", bufs=2) as gp,
            tc.tile_pool(name="g2p", bufs=2) as g2p,
            tc.tile_pool(name="sp", bufs=3) as sp,
            tc.tile_pool(name="mp", bufs=2) as mp,
            tc.tile_pool(name="tailp", bufs=2) as tailp,
            tc.tile_pool(name="ps", bufs=2, space="PSUM") as ps,
        ):
            # ---- constants ----
            ident = cst.tile([P, P], BF)
            make_identity(nc, ident[:])
            iota_i = cst.tile([P, P], I16)
            nc.gpsimd.iota(iota_i[:], pattern=[[1, P]], base=0, channel_multiplier=0)
            iota_bf = cst.tile([P, P], BF)
            nc.vector.tensor_copy(iota_bf[:], iota_i[:])
            iota_rep = cst.tile([P, NSUB, P], BF)
            nc.vector.tensor_copy(
                iota_rep[:], iota_bf[:, None, :].to_broadcast([P, NSUB, P]))
            onesk = cst.tile([1, P], F32)
            nc.vector.memset(onesk[:], 1.0)

            w1e_sb = cst.tile([FIN, 272], BF)
            nc.sync.dma_start(out=w1e_sb[:], in_=w1e_d[:])
            w2e_sb = cst.tile([P, 2, 4], BF)
            nc.sync.dma_start(out=w2e_sb[:], in_=w2e_d[:].rearrange("p (k n) -> p k n", k=2))
            idx_sb = cst.tile([P, NB2 * IDXW], I16)
            nc.sync.dma_start(out=idx_sb[:], in_=idx_d[:])
            idx2_sb = cst.tile([P, NB2 * IDXW], I16)
            nc.sync.dma_start(out=idx2_sb[:], in_=idx2_d[:])
            dst_sb = cst.tile([P, NB2 * NSUB], BF)
            nc.sync.dma_start(out=dst_sb[:], in_=dst_d[:])
            dst2_sb = cst.tile([P, NB2 * NSUB], BF)
            nc.sync.dma_start(out=dst2_sb[:], in_=dst2_d[:])

            # bias broadcast rows -> [P, HD], [P, 2]
            b1r = cst.tile([1, HD], F32)
            nc.sync.dma_start(out=b1r[:], in_=b1_d[:])
            b2r = cst.tile([1, 2], F32)
            nc.sync.dma_start(out=b2r[:], in_=b2_d[:])
            bps = ps.tile([P, HD], F32, space="PSUM", tag="aexp")
            nc.tensor.matmul(out=bps[:], lhsT=onesk[:], rhs=b1r[:], start=True, stop=True)
            b1bc = cst.tile([P, HD], F32)
            nc.scalar.copy(b1bc[:], bps[:])
            bps2 = ps.tile([P, 2], F32, space="PSUM", tag="aexp")
            nc.tensor.matmul(out=bps2[:], lhsT=onesk[:], rhs=b2r[:], start=True, stop=True)
            b2bc = cst.tile([P, 2], F32)
            nc.scalar.copy(b2bc[:], bps2[:])

            adst_sb = cst.tile([P, NBLK, H], BF)
            adst2_sb = cst.tile([P, NBLK], BF)
            outstage = cst.tile([P, NBLK, 2], F32)

            # prime gather buffers so skipped (-1) slots read finite data
            for _ in range(2):
                gz = gp.tile([P, NSUB, ELEM], BF, tag="g")
                nc.vector.memset(gz[:], 0.0)
                g2z = g2p.tile([P, NSUB, ELEM2], BF, tag="g2")
                nc.vector.memset(g2z[:], 0.0)

            # ---- P1: node features -> table (replicated over all nodes) ----
            for ch in range(NPAD // XCHUNK):
                xc = xp.tile([FIN, XCHUNK], BF, tag="xc")
                nc.sync.dma_start(out=xc[:], in_=xT_d[:, ch * XCHUNK:(ch + 1) * XCHUNK])
                for j in range(XCHUNK // P):
                    nt = ch * (XCHUNK // P) + j
                    ph = ps.tile([P, 272], F32, space="PSUM", tag="accum")
                    nc.tensor.matmul(out=ph[:], lhsT=xc[:, j * P:(j + 1) * P],
                                     rhs=w1e_sb[:], start=True, stop=True)
                    row = rowp.tile([P, 272], BF, tag="row")
                    if nt % 2 == 0:
                        nc.scalar.copy(row[:], ph[:])
                    else:
                        nc.vector.tensor_copy(row[:], ph[:])
                    nc.sync.dma_start(out=table[nt * P:(nt + 1) * P, 0:272], in_=row[:])

            # ---- adst slice for own dst range (pid ladder) ----
            pid = nc.sync.partition_id()
            for c in range(NCORES):
                with tc.If(pid == c):
                    nc.sync.dma_start(
                        out=adst_sb[:],
                        in_=table[c * PER:(c + 1) * PER, 264:272]
                            .rearrange("(b p) h -> p b h", p=P))

            # =========== P2: layer-1 message passing (pipelined) ===========
            state = {}
            pblks = {}

            def prep2(k):
                b, half = k // 2, k % 2
                g = gp.tile([P, NSUB, ELEM], BF, tag="g")
                nc.gpsimd.dma_gather(
                    out_ap=g[:],
                    in_ap=(table[0:HALF, :] if half == 0 else table[HALF:NPAD, :]),
                    idxs_ap=idx_sb[:, k * IDXW:(k + 1) * IDXW],
                    num_idxs=NEH, num_idxs_reg=NEH, elem_size=ELEM,
                    single_packet=SP, queue_num=k % NSWQ)
                S = sp.tile([P, NSUB, P], BF, tag="S")
                nc.vector.tensor_tensor(
                    out=S[:], in0=iota_rep[:],
                    in1=dst_sb[:, k * NSUB:(k + 1) * NSUB][:, :, None]
                        .to_broadcast([P, NSUB, P]),
                    op=mybir.AluOpType.is_equal)
                ST = sp.tile([P, NSUB, P], BF, tag="ST", bufs=2)
                for grp in range(2):
                    tps = ps.tile([P, 5, P], BF, space="PSUM", tag="tps")
                    for tt in range(5):
                        t = grp * 5 + tt
                        nc.tensor.transpose(tps[:, tt, :], S[:, t, :], ident[:])
                    nc.scalar.copy(ST[:, grp * 5:(grp + 1) * 5, :], tps[:])
                aexp = ps.tile([P, NSUB, H], F32, space="PSUM", tag="aexp")
                for t in range(NSUB):
                    nc.tensor.matmul(out=aexp[:, t, :], lhsT=ST[:, t, :],
                                     rhs=adst_sb[:, b, :], start=True, stop=True)
                logits = mp.tile([P, NSUB, H], F32, tag="logits")
                nc.vector.tensor_tensor(out=logits[:], in0=g[:, :, 256:264],
                                        in1=aexp[:], op=mybir.AluOpType.add)
                e1 = mp.tile([P, NSUB, H], F32, tag="e1")
                nc.scalar.activation(e1[:], logits[:], mybir.ActivationFunctionType.Exp)
                e2 = mp.tile([P, NSUB, H], F32, tag="e2")
                nc.scalar.activation(e2[:], logits[:], mybir.ActivationFunctionType.Exp,
                                     scale=0.2)
                wt = mp.tile([P, NSUB, H], F32, tag="wt")
                nc.vector.tensor_tensor(out=wt[:], in0=e1[:], in1=e2[:],
                                        op=mybir.AluOpType.max)
                msg = mp.tile([P, NSUB, 264], BF, tag="msg")
                nc.vector.tensor_tensor(
                    out=msg[:, :, 0:256].rearrange("p t (h c) -> p t h c", h=H),
                    in0=g[:, :, 0:256].rearrange("p t (h c) -> p t h c", h=H),
                    in1=wt[:, :, :, None].to_broadcast([P, NSUB, H, C]),
                    op=mybir.AluOpType.mult)
                nc.vector.tensor_copy(msg[:, :, 256:264], wt[:])
                state[k] = (S, msg)

            def agg2(k):
                b, half = k // 2, k % 2
                S, msg = state.pop(k)
                if half == 0:
                    pblks[b] = ps.tile([P, 264], F32, space="PSUM", tag="accum", name="pblk")
                pblk = pblks[b]
                for t in range(NSUB):
                    nc.tensor.matmul(out=pblk[:], lhsT=S[:, t, :], rhs=msg[:, t, :],
                                     start=(half == 0 and t == 0),
                                     stop=(half == 1 and t == NSUB - 1))

            def tail1(b):
                pb = pblks.pop(b)
                srec = tailp.tile([P, H], F32, tag="srec")
                nc.vector.tensor_scalar(
                    out=srec[:], in0=pb[:, 256:264], scalar1=1e-16, scalar2=None,
                    op0=mybir.AluOpType.add)
                rec = tailp.tile([P, H], F32, tag="rec")
                nc.vector.reciprocal(rec[:], srec[:])
                out1 = tailp.tile([P, HD], F32, tag="out1")
                nc.vector.tensor_tensor(
                    out=out1[:].rearrange("p (h c) -> p h c", h=H),
                    in0=pb[:, 0:256].rearrange("p (h c) -> p h c", h=H),
                    in1=rec[:, :, None].to_broadcast([P, H, C]),
                    op=mybir.AluOpType.mult)
                v = tailp.tile([P, HD], F32, tag="v")
                nc.vector.tensor_tensor(out=v[:], in0=out1[:], in1=b1bc[:],
                                        op=mybir.AluOpType.add)
                ev = tailp.tile([P, HD], F32, tag="ev")
                nc.scalar.activation(ev[:], v[:], mybir.ActivationFunctionType.Exp)
                em = tailp.tile([P, HD], F32, tag="em")
                nc.vector.tensor_scalar(out=em[:], in0=ev[:], scalar1=1.0, scalar2=0.0,
                                        op0=mybir.AluOpType.subtract,
                                        op1=mybir.AluOpType.min)
                pp = tailp.tile([P, HD], F32, tag="pp")
                nc.vector.tensor_scalar(out=pp[:], in0=v[:], scalar1=0.0, scalar2=None,
                                        op0=mybir.AluOpType.max)
                elu = tailp.tile([P, HD], BF, tag="elu")
                nc.vector.tensor_tensor(out=elu[:], in0=em[:], in1=pp[:],
                                        op=mybir.AluOpType.add)
                eT_ps = ps.tile([P, 2, P], BF, space="PSUM", tag="tps")
                for kk in range(2):
                    nc.tensor.transpose(eT_ps[:, kk, :], elu[:, kk * P:(kk + 1) * P],
                                        ident[:])
                eT_sb = tailp.tile([P, 2, P], BF, tag="eT")
                nc.scalar.copy(eT_sb[:], eT_ps[:])
                ph2 = ps.tile([P, 4], F32, space="PSUM", tag="aexp")
                for kk in range(2):
                    nc.tensor.matmul(out=ph2[:], lhsT=eT_sb[:, kk, :],
                                     rhs=w2e_sb[:, kk, :],
                                     start=(kk == 0), stop=(kk == 1))
                t2row = tailp.tile([P, 4], BF, tag="t2row")
                nc.vector.tensor_copy(t2row[:], ph2[:, 0:4])
                nc.vector.tensor_copy(adst2_sb[:, b:b + 1], ph2[:, 3:4])
                kag = b // CBLK
                rb = (b % CBLK) * P
                nc.sync.dma_start(out=t2locs[kag][rb:rb + P, 0:4], in_=t2row[:])

            for k in range(NB2 + 1):
                if k >= 1:
                    agg2(k - 1)
                    if (k - 1) % 2 == 1:
                        b = (k - 1) // 2
                        tail1(b)
                        if (b + 1) % CBLK == 0:
                            kag = b // CBLK
                            nc.gpsimd.collective_compute(
                                "AllGather", mybir.AluOpType.bypass,
                                replica_groups=[list(range(NCORES))],
                                ins=[t2locs[kag][:]],
                                outs=[table2[kag * NCORES * CROWS:
                                             (kag + 1) * NCORES * CROWS, :]])
                if k < NB2:
                    prep2(k)

            # =========== P3: layer-2 message passing (pipelined) ===========
            def prep3(k):
                b, half = k // 2, k % 2
                g2 = g2p.tile([P, NSUB, ELEM2], BF, tag="g2")
                nc.gpsimd.dma_gather(
                    out_ap=g2[:],
                    in_ap=(table2[0:HALF, :] if half == 0 else table2[HALF:NPAD, :]),
                    idxs_ap=idx2_sb[:, k * IDXW:(k + 1) * IDXW],
                    num_idxs=NEH, num_idxs_reg=NEH, elem_size=ELEM2,
                    single_packet=SP, queue_num=k % NSWQ)
                S = sp.tile([P, NSUB, P], BF, tag="S")
                nc.vector.tensor_tensor(
                    out=S[:], in0=iota_rep[:],
                    in1=dst2_sb[:, k * NSUB:(k + 1) * NSUB][:, :, None]
                        .to_broadcast([P, NSUB, P]),
                    op=mybir.AluOpType.is_equal)
                ST = sp.tile([P, NSUB, P], BF, tag="ST", bufs=2)
                for grp in range(2):
                    tps = ps.tile([P, 5, P], BF, space="PSUM", tag="tps")
                    for tt in range(5):
                        t = grp * 5 + tt
                        nc.tensor.transpose(tps[:, tt, :], S[:, t, :], ident[:])
                    nc.scalar.copy(ST[:, grp * 5:(grp + 1) * 5, :], tps[:])
                a2e = ps.tile([P, NSUB], F32, space="PSUM", tag="aexp")
                for t in range(NSUB):
                    nc.tensor.matmul(out=a2e[:, t:t + 1], lhsT=ST[:, t, :],
                                     rhs=adst2_sb[:, b:b + 1], start=True, stop=True)
                lg2 = mp.tile([P, NSUB], F32, tag="lg2")
                nc.vector.tensor_tensor(
                    out=lg2[:], in0=g2[:, :, 2:3].rearrange("p t x -> p (t x)"),
                    in1=a2e[:], op=mybir.AluOpType.add)
                f1 = mp.tile([P, NSUB], F32, tag="f1")
                nc.scalar.activation(f1[:], lg2[:], mybir.ActivationFunctionType.Exp)
                f2 = mp.tile([P, NSUB], F32, tag="f2")
                nc.scalar.activation(f2[:], lg2[:], mybir.ActivationFunctionType.Exp,
                                     scale=0.2)
                w2t = mp.tile([P, NSUB], F32, tag="w2t")
                nc.vector.tensor_tensor(out=w2t[:], in0=f1[:], in1=f2[:],
                                        op=mybir.AluOpType.max)
                msg2 = mp.tile([P, NSUB, 3], BF, tag="msg2")
                nc.vector.tensor_tensor(
                    out=msg2[:, :, 0:2], in0=g2[:, :, 0:2],
                    in1=w2t[:, :, None].to_broadcast([P, NSUB, 2]),
                    op=mybir.AluOpType.mult)
                nc.vector.tensor_copy(msg2[:, :, 2:3], w2t[:, :, None])
                state[k] = (S, msg2)

            def agg3(k):
                b, half = k // 2, k % 2
                S, msg2 = state.pop(k)
                if half == 0:
                    pblks[b] = ps.tile([P, 3], F32, space="PSUM", tag="accum", name="p2s")
                p2s = pblks[b]
                for t in range(NSUB):
                    nc.tensor.matmul(out=p2s[:], lhsT=S[:, t, :], rhs=msg2[:, t, :],
                                     start=(half == 0 and t == 0),
                                     stop=(half == 1 and t == NSUB - 1))

            def tail3(b):
                p2s = pblks.pop(b)
                s2r = tailp.tile([P, 1], F32, tag="s2r")
                nc.vector.tensor_scalar(out=s2r[:], in0=p2s[:, 2:3], scalar1=1e-16,
                                        scalar2=None, op0=mybir.AluOpType.add)
                rec2 = tailp.tile([P, 1], F32, tag="rec2")
                nc.vector.reciprocal(rec2[:], s2r[:])
                o2 = tailp.tile([P, 2], F32, tag="o2")
                nc.vector.tensor_tensor(out=o2[:], in0=p2s[:, 0:2],
                                        in1=rec2[:].to_broadcast([P, 2]),
                                        op=mybir.AluOpType.mult)
                nc.vector.tensor_tensor(out=outstage[:, b, :], in0=o2[:], in1=b2bc[:],
                                        op=mybir.AluOpType.add)

            for k in range(NB2 + 1):
                if k >= 1:
                    agg3(k - 1)
                    if (k - 1) % 2 == 1:
                        tail3((k - 1) // 2)
                if k < NB2:
                    prep3(k)

            nc.sync.dma_start(
                out=out_d[:].rearrange("(b p) c -> p b c", p=P), in_=outstage[:])

    nc.compile()
    return nc


def host_prep(inputs, cfg):
    """Build per-core input maps from full inputs."""
    N, NPAD, PER, NBLK, HALF, NSUB, NCORES, NAG = (
        cfg["N"], cfg["NPAD"], cfg["PER"], cfg["NBLK"], cfg["HALF"],
        cfg["NSUB"], cfg["NCORES"], cfg["NAG"])
    NEH = NSUB * P
    IDXW = NEH // 16
    CBLK = NBLK // NAG

    x = np.asarray(inputs["x"], dtype=np.float32)
    ei = np.asarray(inputs["edge_index"], dtype=np.int64)
    W1 = np.asarray(inputs["W1"], dtype=np.float64)
    a1s = np.asarray(inputs["a1_src"], dtype=np.float64)
    a1d = np.asarray(inputs["a1_dst"], dtype=np.float64)
    b1 = np.asarray(inputs["b1"], dtype=np.float32)
    W2 = np.asarray(inputs["W2"], dtype=np.float64)
    a2s = np.asarray(inputs["a2_src"], dtype=np.float64)
    a2d = np.asarray(inputs["a2_dst"], dtype=np.float64)
    b2 = np.asarray(inputs["b2"], dtype=np.float32)

    xT = np.zeros((FIN, NPAD), dtype=np.float32)
    xT[:, :N] = x.T
    xT = xT.astype(NP_BF)

    A1s = np.zeros((HD, H))
    A1d = np.zeros((HD, H))
    for hd in range(H):
        A1s[hd * C:(hd + 1) * C, hd] = a1s[hd]
        A1d[hd * C:(hd + 1) * C, hd] = a1d[hd]
    w1e = np.concatenate([W1, W1 @ A1s, W1 @ A1d], axis=1).astype(NP_BF)  # [128,272]

    w2cols = np.concatenate([W2, W2 @ a2s[0][:, None], W2 @ a2d[0][:, None]],
                            axis=1)  # [HD, 4]
    w2e = w2cols.reshape(2, P, 4).transpose(1, 0, 2).reshape(P, 8).astype(NP_BF)

    loops = np.arange(N, dtype=np.int64)
    src = np.concatenate([ei[0], loops])
    dst = np.concatenate([ei[1], loops])

    # chunk-major table2 row index for each node
    nodes = np.arange(NPAD, dtype=np.int64)
    n_c, n_r = nodes // PER, nodes % PER
    n_b, n_j = n_r >> 7, n_r & 127
    n_k = n_b // CBLK
    row2_of = (n_k * (NCORES * CBLK * P) + n_c * (CBLK * P)
               + (n_b - n_k * CBLK) * P + n_j)

    in_maps = []
    for c in range(NCORES):
        lo_n, hi_n = c * PER, (c + 1) * PER
        m = (dst >= lo_n) & (dst < hi_n)
        s_c = src[m]
        d_c = dst[m] - lo_n
        blk = d_c >> 7
        dloc = d_c & 127

        packs = []
        for srow in (s_c, row2_of[s_c]):
            halfsel = (srow >= HALF).astype(np.int64)
            key = blk * 2 + halfsel
            order = np.argsort(key, kind="stable")
            key_s = key[order]
            cnt = np.bincount(key_s, minlength=NBLK * 2)
            assert cnt.max() <= NEH, f"bucket overflow: {cnt.max()} > {NEH}"
            starts = np.zeros(NBLK * 2, dtype=np.int64)
            starts[1:] = np.cumsum(cnt)[:-1]
            pos = np.arange(len(key_s)) - starts[key_s]
            slot = key_s * NEH + pos
            pad_idx = -1 if cfg.get("NEG", 1) else 0
            idxflat = np.full(NBLK * 2 * NEH, pad_idx, dtype=np.int16)
            dstflat = np.full(NBLK * 2 * NEH, -1.0, dtype=np.float32)
            sv = srow[order] - halfsel[order] * HALF
            idxflat[slot] = sv.astype(np.int16)
            dstflat[slot] = dloc[order].astype(np.float32)
            idxw16 = (idxflat.reshape(NBLK * 2, NSUB * 8, 16)
                      .transpose(2, 0, 1).reshape(16, -1))
            idxw = np.tile(idxw16, (8, 1))  # replicated across the 8 Q7 cores
            dstw = (dstflat.reshape(NBLK * 2, NSUB, P).transpose(2, 0, 1)
                    .reshape(P, NBLK * 2 * NSUB)).astype(NP_BF)
            packs.append((idxw, np.ascontiguousarray(dstw)))

        in_maps.append({
            "xT": xT, "w1e": w1e, "w2e": w2e,
            "b1r": b1.reshape(1, HD).astype(np.float32),
            "b2r": b2.reshape(1, 2).astype(np.float32),
            "idx16": packs[0][0], "dstf": packs[0][1],
            "idx16b": packs[1][0], "dstfb": packs[1][1],
        })
    return in_maps


_NC_CACHE = {}


def _get_nc():
    if "nc" not in _NC_CACHE:
        _NC_CACHE["nc"] = build_nc(FULL_CFG)
    return _NC_CACHE["nc"]


def kernel(**inputs):
    from concourse.bass_utils import run_bass_kernel_spmd

    nc = _get_nc()
    in_maps = host_prep(inputs, FULL_CFG)
    res = run_bass_kernel_spmd(nc, in_maps, core_ids=list(range(FULL_CFG["NCORES"])))
    out = np.concatenate([r["out"] for r in res.results])[:FULL_CFG["N"]]
    return np.ascontiguousarray(out.astype(np.float32))


# revision 16
# speedup vs baseline: 1.2534x; 1.0582x over previous
"""Self-contained Trainium2 Bass kernel for the 2-layer GAT problem.

Accepts FULL inputs, shards destination-node ranges across 8 NeuronCores
internally, and returns the FULL [50000, 2] float32 output.

Structure (per core):
  P1: replicated node transform x@[W1|W1@A1s|W1@A1d] -> DRAM table rows
      [h(256) | asrc(8) | adst(8)] bf16, padded to 384-col rows (768B gather
      elems). adst captured into SBUF on the fly.
  P2: per (dst-block, src-half) bucket of <=1280 edges: dma_gather source
      rows (skip -1 pads), batched one-hot S build, PE-transpose -> ST,
      per-subtile adst broadcast via tiny matmuls, leakyrelu-softmax weights
      via exp/exp(0.2x)/max, aggregation matmuls with the weight column
      appended to the rhs (denominator accumulates in PSUM cols 256:264).
      Software-pipelined with a 1-bucket lookahead.
  AllGather of the layer-2 node table in 7 chunks (overlapped under P2),
      chunk-major table2 layout so each chunk is a contiguous AG output.
  P3: same machinery on the 4-wide layer-2 rows (256B gather elems).
"""
import numpy as np

import concourse.bacc as bacc
import concourse.mybir as mybir
import concourse.tile as tile
from concourse.masks import make_identity

F32 = mybir.dt.float32
BF = mybir.dt.bfloat16
I16 = mybir.dt.int16
NP_BF = mybir.dt.np(BF)

H = 8       # heads
C = 32      # per-head channels
HD = H * C  # 256
FIN = 128
ELEM = 384   # table row elems (768B); cols 0:272 used
ELEM2 = 128  # table2 row elems (256B); cols 0:4 used
P = 128

import os as _os

FULL_CFG = dict(
    N=50000, NPAD=50176, PER=6272, NBLK=49, HALF=25088, NSUB=10, NCORES=8,
    XCHUNK=1024,
    NAG=int(_os.environ.get("GAT_NAG", "7")),
    NEG=int(_os.environ.get("GAT_NEG", "1")),
    NSWQ=int(_os.environ.get("GAT_NSWQ", "1")),
    SP=int(_os.environ.get("GAT_SP", "0")),
)


def build_nc(cfg):
    NPAD, PER, NBLK, HALF, NSUB = (
        cfg["NPAD"], cfg["PER"], cfg["NBLK"], cfg["HALF"], cfg["NSUB"])
    NCORES = cfg["NCORES"]
    XCHUNK = cfg["XCHUNK"]
    NAG = cfg["NAG"]
    NEH = NSUB * P                # idxs per (block, half) gather
    IDXW = NEH // 16              # idx cols per bucket
    NTILE = NPAD // P             # node tiles in P1
    NB2 = NBLK * 2                # buckets
    CBLK = NBLK // NAG            # blocks per AG chunk
    CROWS = CBLK * P              # local rows per AG chunk
    assert NPAD == NCORES * PER and PER == NBLK * P and NPAD % XCHUNK == 0
    assert HALF % P == 0 and 2 * HALF == NPAD and NBLK == NAG * CBLK

    NSWQ = cfg.get("NSWQ", 1)
    SP = bool(cfg.get("SP", 0))
    NEG = bool(cfg.get("NEG", 1))
    nc = bacc.Bacc(None, target_bir_lowering=False, num_devices=NCORES,
                   num_swdge_queues=NSWQ)

    xT_d = nc.dram_tensor("xT", [FIN, NPAD], BF, kind="ExternalInput")
    w1e_d = nc.dram_tensor("w1e", [FIN, 272], BF, kind="ExternalInput")
    w2e_d = nc.dram_tensor("w2e", [P, 8], BF, kind="ExternalInput")
    b1_d = nc.dram_tensor("b1r", [1, HD], F32, kind="ExternalInput")
    b2_d = nc.dram_tensor("b2r", [1, 2], F32, kind="ExternalInput")
    idx_d = nc.dram_tensor("idx16", [P, NB2 * IDXW], I16, kind="ExternalInput")
    idx2_d = nc.dram_tensor("idx16b", [P, NB2 * IDXW], I16, kind="ExternalInput")
    dst_d = nc.dram_tensor("dstf", [P, NB2 * NSUB], BF, kind="ExternalInput")
    dst2_d = nc.dram_tensor("dstfb", [P, NB2 * NSUB], BF, kind="ExternalInput")
    cnt_d = nc.dram_tensor("cnts", [1, NB2], mybir.dt.int32, kind="ExternalInput")
    cnt2_d = nc.dram_tensor("cnts2", [1, NB2], mybir.dt.int32, kind="ExternalInput")
    out_d = nc.dram_tensor("out", [PER, 2], F32, kind="ExternalOutput")

    table = nc.dram_tensor("table", [NPAD, ELEM], BF)
    t2locs = [nc.dram_tensor(f"t2loc{k}", [CROWS, ELEM2], BF) for k in range(NAG)]
    table2 = nc.dram_tensor("table2", [NPAD, ELEM2], BF)

    with tile.TileContext(nc) as tc:
        with (
            tc.tile_pool(name="cst", bufs=1) as cst,
            tc.tile_pool(name="xp", bufs=2) as xp,
            tc.tile_pool(name="rowp", bufs=3) as rowp,
            tc.tile_pool(name="gp", bufs=3) as gp,
            tc.tile_pool(name="g2p", bufs=3) as g2p,
            tc.tile_pool(name="sp", bufs=3) as sp,
            tc.tile_pool(name="mp", bufs=3) as mp,
            tc.tile_pool(name="tailp", bufs=2) as tailp,
            tc.tile_pool(name="ps", bufs=2, space="PSUM") as ps,
        ):
            # ---- constants ----
            ident = cst.tile([P, P], BF)
            make_identity(nc, ident[:])
            iota_i = cst.tile([P, P], I16)
            nc.gpsimd.iota(iota_i[:], pattern=[[1, P]], base=0, channel_multiplier=0)
            iota_bf = cst.tile([P, P], BF)
            nc.vector.tensor_copy(iota_bf[:], iota_i[:])
            iota_rep = cst.tile([P, NSUB, P], BF)
            nc.vector.tensor_copy(
                iota_rep[:], iota_bf[:, None, :].to_broadcast([P, NSUB, P]))
            onesk = cst.tile([1, P], F32)
            nc.vector.memset(onesk[:], 1.0)

            w1e_sb = cst.tile([FIN, 272], BF)
            nc.sync.dma_start(out=w1e_sb[:], in_=w1e_d[:])
            w2e_sb = cst.tile([P, 2, 4], BF)
            nc.sync.dma_start(out=w2e_sb[:], in_=w2e_d[:].rearrange("p (k n) -> p k n", k=2))
            idx_sb = cst.tile([P, NB2 * IDXW], I16)
            nc.sync.dma_start(out=idx_sb[:], in_=idx_d[:])
            idx2_sb = cst.tile([P, NB2 * IDXW], I16)
            nc.sync.dma_start(out=idx2_sb[:], in_=idx2_d[:])
            dst_sb = cst.tile([P, NB2 * NSUB], BF)
            nc.sync.dma_start(out=dst_sb[:], in_=dst_d[:])
            dst2_sb = cst.tile([P, NB2 * NSUB], BF)
            nc.sync.dma_start(out=dst2_sb[:], in_=dst2_d[:])
            cnt_sb = cst.tile([1, NB2], mybir.dt.int32)
            nc.sync.dma_start(out=cnt_sb[:], in_=cnt_d[:])
            cnt2_sb = cst.tile([1, NB2], mybir.dt.int32)
            nc.sync.dma_start(out=cnt2_sb[:], in_=cnt2_d[:])

            # bias broadcast rows -> [P, HD], [P, 2]
            b1r = cst.tile([1, HD], F32)
            nc.sync.dma_start(out=b1r[:], in_=b1_d[:])
            b2r = cst.tile([1, 2], F32)
            nc.sync.dma_start(out=b2r[:], in_=b2_d[:])
            bps = ps.tile([P, HD], F32, space="PSUM", tag="aexp")
            nc.tensor.matmul(out=bps[:], lhsT=onesk[:], rhs=b1r[:], start=True, stop=True)
            b1bc = cst.tile([P, HD], F32)
            nc.scalar.copy(b1bc[:], bps[:])
            bps2 = ps.tile([P, 2], F32, space="PSUM", tag="aexp")
            nc.tensor.matmul(out=bps2[:], lhsT=onesk[:], rhs=b2r[:], start=True, stop=True)
            b2bc = cst.tile([P, 2], F32)
            nc.scalar.copy(b2bc[:], bps2[:])

            adst_sb = cst.tile([P, NBLK, H], BF)
            adst2_sb = cst.tile([P, NBLK], BF)
            outstage = cst.tile([P, NBLK, 2], F32)

            # prime gather buffers so skipped (-1) slots read finite data
            for _ in range(3):
                gz = gp.tile([P, NSUB, ELEM], BF, tag="g")
                nc.vector.memset(gz[:], 0.0)
                g2z = g2p.tile([P, NSUB, ELEM2], BF, tag="g2")
                nc.vector.memset(g2z[:], 0.0)

            # ---- P1: node features -> table (replicated over all nodes) ----
            for ch in range(NPAD // XCHUNK):
                xc = xp.tile([FIN, XCHUNK], BF, tag="xc")
                nc.sync.dma_start(out=xc[:], in_=xT_d[:, ch * XCHUNK:(ch + 1) * XCHUNK])
                for j in range(XCHUNK // P):
                    nt = ch * (XCHUNK // P) + j
                    ph = ps.tile([P, 272], F32, space="PSUM", tag="accum")
                    nc.tensor.matmul(out=ph[:], lhsT=xc[:, j * P:(j + 1) * P],
                                     rhs=w1e_sb[:], start=True, stop=True)
                    row = rowp.tile([P, 272], BF, tag="row")
                    if nt % 2 == 0:
                        nc.scalar.copy(row[:], ph[:])
                    else:
                        nc.vector.tensor_copy(row[:], ph[:])
                    nc.sync.dma_start(out=table[nt * P:(nt + 1) * P, 0:272], in_=row[:])

            # ---- adst slice for own dst range (pid ladder) ----
            pid = nc.sync.partition_id()
            for c in range(NCORES):
                with tc.If(pid == c):
                    nc.sync.dma_start(
                        out=adst_sb[:],
                        in_=table[c * PER:(c + 1) * PER, 264:272]
                            .rearrange("(b p) h -> p b h", p=P))

            # =========== P2: layer-1 message passing (pipelined) ===========
            state = {}
            pblks = {}

            def prep2(k):
                b, half = k // 2, k % 2
                g = gp.tile([P, NSUB, ELEM], BF, tag="g")
                if NEG:
                    creg = nc.gpsimd.alloc_register()
                    nc.gpsimd.reg_load(creg, cnt_sb[:, k:k + 1])
                    nreg = creg
                else:
                    nreg = NEH
                nc.gpsimd.dma_gather(
                    out_ap=g[:],
                    in_ap=(table[0:HALF, :] if half == 0 else table[HALF:NPAD, :]),
                    idxs_ap=idx_sb[:, k * IDXW:(k + 1) * IDXW],
                    num_idxs=NEH, num_idxs_reg=nreg, elem_size=ELEM,
                    single_packet=SP, queue_num=k % NSWQ)
                S = sp.tile([P, NSUB, P], BF, tag="S")
                nc.vector.tensor_tensor(
                    out=S[:], in0=iota_rep[:],
                    in1=dst_sb[:, k * NSUB:(k + 1) * NSUB][:, :, None]
                        .to_broadcast([P, NSUB, P]),
                    op=mybir.AluOpType.is_equal)
                ST = sp.tile([P, NSUB, P], BF, tag="ST", bufs=2)
                for grp in range(2):
                    tps = ps.tile([P, 5, P], BF, space="PSUM", tag="tps")
                    for tt in range(5):
                        t = grp * 5 + tt
                        nc.tensor.transpose(tps[:, tt, :], S[:, t, :], ident[:])
                    nc.scalar.copy(ST[:, grp * 5:(grp + 1) * 5, :], tps[:])
                aexp = ps.tile([P, NSUB, H], F32, space="PSUM", tag="aexp")
                for t in range(NSUB):
                    nc.tensor.matmul(out=aexp[:, t, :], lhsT=ST[:, t, :],
                                     rhs=adst_sb[:, b, :], start=True, stop=True)
                logits = mp.tile([P, NSUB, H], F32, tag="logits")
                nc.vector.tensor_tensor(out=logits[:], in0=g[:, :, 256:264],
                                        in1=aexp[:], op=mybir.AluOpType.add)
                e1 = mp.tile([P, NSUB, H], F32, tag="e1")
                nc.scalar.activation(e1[:], logits[:], mybir.ActivationFunctionType.Exp)
                e2 = mp.tile([P, NSUB, H], F32, tag="e2")
                nc.scalar.activation(e2[:], logits[:], mybir.ActivationFunctionType.Exp,
                                     scale=0.2)
                wt = mp.tile([P, NSUB, H], F32, tag="wt")
                nc.vector.tensor_tensor(out=wt[:], in0=e1[:], in1=e2[:],
                                        op=mybir.AluOpType.max)
                msg = mp.tile([P, NSUB, 264], BF, tag="msg")
                nc.vector.tensor_tensor(
                    out=msg[:, :, 0:256].rearrange("p t (h c) -> p t h c", h=H),
                    in0=g[:, :, 0:256].rearrange("p t (h c) -> p t h c", h=H),
                    in1=wt[:, :, :, None].to_broadcast([P, NSUB, H, C]),
                    op=mybir.AluOpType.mult)
                nc.vector.tensor_copy(msg[:, :, 256:264], wt[:])
                state[k] = (S, msg)

            def agg2(k):
                b, half = k // 2, k % 2
                S, msg = state.pop(k)
                if half == 0:
                    pblks[b] = ps.tile([P, 264], F32, space="PSUM", tag="accum", name="pblk")
                pblk = pblks[b]
                for t in range(NSUB):
                    nc.tensor.matmul(out=pblk[:], lhsT=S[:, t, :], rhs=msg[:, t, :],
                                     start=(half == 0 and t == 0),
                                     stop=(half == 1 and t == NSUB - 1))

            def tail1(b):
                pb = pblks.pop(b)
                srec = tailp.tile([P, H], F32, tag="srec")
                nc.vector.tensor_scalar(
                    out=srec[:], in0=pb[:, 256:264], scalar1=1e-16, scalar2=None,
                    op0=mybir.AluOpType.add)
                rec = tailp.tile([P, H], F32, tag="rec")
                nc.vector.reciprocal(rec[:], srec[:])
                out1 = tailp.tile([P, HD], F32, tag="out1")
                nc.vector.tensor_tensor(
                    out=out1[:].rearrange("p (h c) -> p h c", h=H),
                    in0=pb[:, 0:256].rearrange("p (h c) -> p h c", h=H),
                    in1=rec[:, :, None].to_broadcast([P, H, C]),
                    op=mybir.AluOpType.mult)
                v = tailp.tile([P, HD], F32, tag="v")
                nc.vector.tensor_tensor(out=v[:], in0=out1[:], in1=b1bc[:],
                                        op=mybir.AluOpType.add)
                ev = tailp.tile([P, HD], F32, tag="ev")
                nc.scalar.activation(ev[:], v[:], mybir.ActivationFunctionType.Exp)
                em = tailp.tile([P, HD], F32, tag="em")
                nc.vector.tensor_scalar(out=em[:], in0=ev[:], scalar1=1.0, scalar2=0.0,
                                        op0=mybir.AluOpType.subtract,
                                        op1=mybir.AluOpType.min)
                pp = tailp.tile([P, HD], F32, tag="pp")
                nc.vector.tensor_scalar(out=pp[:], in0=v[:], scalar1=0.0, scalar2=None,
                                        op0=mybir.AluOpType.max)
                elu = tailp.tile([P, HD], BF, tag="elu")
                nc.vector.tensor_tensor(out=elu[:], in0=em[:], in1=pp[:],
                                        op=mybir.AluOpType.add)
                eT_ps = ps.tile([P, 2, P], BF, space="PSUM", tag="tps")
                for kk in range(2):
                    nc.tensor.transpose(eT_ps[:, kk, :], elu[:, kk * P:(kk + 1) * P],
                                        ident[:])
                eT_sb = tailp.tile([P, 2, P], BF, tag="eT")
                nc.scalar.copy(eT_sb[:], eT_ps[:])
                ph2 = ps.tile([P, 4], F32, space="PSUM", tag="aexp")
                for kk in range(2):
                    nc.tensor.matmul(out=ph2[:], lhsT=eT_sb[:, kk, :],
                                     rhs=w2e_sb[:, kk, :],
                                     start=(kk == 0), stop=(kk == 1))
                t2row = tailp.tile([P, 4], BF, tag="t2row")
                nc.vector.tensor_copy(t2row[:], ph2[:, 0:4])
                nc.vector.tensor_copy(adst2_sb[:, b:b + 1], ph2[:, 3:4])
                kag = b // CBLK
                rb = (b % CBLK) * P
                nc.sync.dma_start(out=t2locs[kag][rb:rb + P, 0:4], in_=t2row[:])

            for k in range(NB2 + 1):
                if k >= 1:
                    agg2(k - 1)
                    if (k - 1) % 2 == 1:
                        b = (k - 1) // 2
                        tail1(b)
                        if (b + 1) % CBLK == 0:
                            kag = b // CBLK
                            nc.gpsimd.collective_compute(
                                "AllGather", mybir.AluOpType.bypass,
                                replica_groups=[list(range(NCORES))],
                                ins=[t2locs[kag][:]],
                                outs=[table2[kag * NCORES * CROWS:
                                             (kag + 1) * NCORES * CROWS, :]])
                if k < NB2:
                    prep2(k)

            # =========== P3: layer-2 message passing (pipelined) ===========
            def prep3(k):
                b, half = k // 2, k % 2
                g2 = g2p.tile([P, NSUB, ELEM2], BF, tag="g2")
                if NEG:
                    creg = nc.gpsimd.alloc_register()
                    nc.gpsimd.reg_load(creg, cnt2_sb[:, k:k + 1])
                    nreg = creg
                else:
                    nreg = NEH
                nc.gpsimd.dma_gather(
                    out_ap=g2[:],
                    in_ap=(table2[0:HALF, :] if half == 0 else table2[HALF:NPAD, :]),
                    idxs_ap=idx2_sb[:, k * IDXW:(k + 1) * IDXW],
                    num_idxs=NEH, num_idxs_reg=nreg, elem_size=ELEM2,
                    single_packet=SP, queue_num=k % NSWQ)
                S = sp.tile([P, NSUB, P], BF, tag="S")
                nc.vector.tensor_tensor(
                    out=S[:], in0=iota_rep[:],
                    in1=dst2_sb[:, k * NSUB:(k + 1) * NSUB][:, :, None]
                        .to_broadcast([P, NSUB, P]),
                    op=mybir.AluOpType.is_equal)
                ST = sp.tile([P, NSUB, P], BF, tag="ST", bufs=2)
                for grp in range(2):
                    tps = ps.tile([P, 5, P], BF, space="PSUM", tag="tps")
                    for tt in range(5):
                        t = grp * 5 + tt
                        nc.tensor.transpose(tps[:, tt, :], S[:, t, :], ident[:])
                    nc.scalar.copy(ST[:, grp * 5:(grp + 1) * 5, :], tps[:])
                a2e = ps.tile([P, NSUB], F32, space="PSUM", tag="aexp")
                for t in range(NSUB):
                    nc.tensor.matmul(out=a2e[:, t:t + 1], lhsT=ST[:, t, :],
                                     rhs=adst2_sb[:, b:b + 1], start=True, stop=True)
                lg2 = mp.tile([P, NSUB], F32, tag="lg2")
                nc.vector.tensor_tensor(
                    out=lg2[:], in0=g2[:, :, 2:3].rearrange("p t x -> p (t x)"),
                    in1=a2e[:], op=mybir.AluOpType.add)
                f1 = mp.tile([P, NSUB], F32, tag="f1")
                nc.scalar.activation(f1[:], lg2[:], mybir.ActivationFunctionType.Exp)
                f2 = mp.tile([P, NSUB], F32, tag="f2")
                nc.scalar.activation(f2[:], lg2[:], mybir.ActivationFunctionType.Exp,
                                     scale=0.2)
                w2t = mp.tile([P, NSUB], F32, tag="w2t")
                nc.vector.tensor_tensor(out=w2t[:], in0=f1[:], in1=f2[:],
                                        op=mybir.AluOpType.max)
                msg2 = mp.tile([P, NSUB, 3], BF, tag="msg2")
                nc.vector.tensor_tensor(
                    out=msg2[:, :, 0:2], in0=g2[:, :, 0:2],
                    in1=w2t[:, :, None].to_broadcast([P, NSUB, 2]),
                    op=mybir.AluOpType.mult)
                nc.vector.tensor_copy(msg2[:, :, 2:3], w2t[:, :, None])
                state[k] = (S, msg2)

            def agg3(k):
                b, half = k // 2, k % 2
                S, msg2 = state.pop(k)
                if half == 0:
                    pblks[b] = ps.tile([P, 3], F32, space="PSUM", tag="accum", name="p2s")
                p2s = pblks[b]
                for t in range(NSUB):
                    nc.tensor.matmul(out=p2s[:], lhsT=S[:, t, :], rhs=msg2[:, t, :],
                                     start=(half == 0 and t == 0),
                                     stop=(half == 1 and t == NSUB - 1))

            def tail3(b):
                p2s = pblks.pop(b)
                s2r = tailp.tile([P, 1], F32, tag="s2r")
                nc.vector.tensor_scalar(out=s2r[:], in0=p2s[:, 2:3], scalar1=1e-16,
                                        scalar2=None, op0=mybir.AluOpType.add)
                rec2 = tailp.tile([P, 1], F32, tag="rec2")
                nc.vector.reciprocal(rec2[:], s2r[:])
                o2 = tailp.tile([P, 2], F32, tag="o2")
                nc.vector.tensor_tensor(out=o2[:], in0=p2s[:, 0:2],
                                        in1=rec2[:].to_broadcast([P, 2]),
                                        op=mybir.AluOpType.mult)
                nc.vector.tensor_tensor(out=outstage[:, b, :], in0=o2[:], in1=b2bc[:],
                                        op=mybir.AluOpType.add)

            for k in range(NB2 + 1):
                if k >= 1:
                    agg3(k - 1)
                    if (k - 1) % 2 == 1:
                        tail3((k - 1) // 2)
                if k < NB2:
                    prep3(k)

            nc.sync.dma_start(
                out=out_d[:].rearrange("(b p) c -> p b c", p=P), in_=outstage[:])

    nc.compile()
    return nc


def host_prep(inputs, cfg):
    """Build per-core input maps from full inputs."""
    N, NPAD, PER, NBLK, HALF, NSUB, NCORES, NAG = (
        cfg["N"], cfg["NPAD"], cfg["PER"], cfg["NBLK"], cfg["HALF"],
        cfg["NSUB"], cfg["NCORES"], cfg["NAG"])
    NEH = NSUB * P
    IDXW = NEH // 16
    CBLK = NBLK // NAG

    x = np.asarray(inputs["x"], dtype=np.float32)
    ei = np.asarray(inputs["edge_index"], dtype=np.int64)
    W1 = np.asarray(inputs["W1"], dtype=np.float64)
    a1s = np.asarray(inputs["a1_src"], dtype=np.float64)
    a1d = np.asarray(inputs["a1_dst"], dtype=np.float64)
    b1 = np.asarray(inputs["b1"], dtype=np.float32)
    W2 = np.asarray(inputs["W2"], dtype=np.float64)
    a2s = np.asarray(inputs["a2_src"], dtype=np.float64)
    a2d = np.asarray(inputs["a2_dst"], dtype=np.float64)
    b2 = np.asarray(inputs["b2"], dtype=np.float32)

    xT = np.zeros((FIN, NPAD), dtype=np.float32)
    xT[:, :N] = x.T
    xT = xT.astype(NP_BF)

    A1s = np.zeros((HD, H))
    A1d = np.zeros((HD, H))
    for hd in range(H):
        A1s[hd * C:(hd + 1) * C, hd] = a1s[hd]
        A1d[hd * C:(hd + 1) * C, hd] = a1d[hd]
    w1e = np.concatenate([W1, W1 @ A1s, W1 @ A1d], axis=1).astype(NP_BF)  # [128,272]

    w2cols = np.concatenate([W2, W2 @ a2s[0][:, None], W2 @ a2d[0][:, None]],
                            axis=1)  # [HD, 4]
    w2e = w2cols.reshape(2, P, 4).transpose(1, 0, 2).reshape(P, 8).astype(NP_BF)

    loops = np.arange(N, dtype=np.int64)
    src = np.concatenate([ei[0], loops])
    dst = np.concatenate([ei[1], loops])

    # chunk-major table2 row index for each node
    nodes = np.arange(NPAD, dtype=np.int64)
    n_c, n_r = nodes // PER, nodes % PER
    n_b, n_j = n_r >> 7, n_r & 127
    n_k = n_b // CBLK
    row2_of = (n_k * (NCORES * CBLK * P) + n_c * (CBLK * P)
               + (n_b - n_k * CBLK) * P + n_j)

    in_maps = []
    for c in range(NCORES):
        lo_n, hi_n = c * PER, (c + 1) * PER
        m = (dst >= lo_n) & (dst < hi_n)
        s_c = src[m]
        d_c = dst[m] - lo_n
        blk = d_c >> 7
        dloc = d_c & 127

        packs = []
        for srow in (s_c, row2_of[s_c]):
            halfsel = (srow >= HALF).astype(np.int64)
            key = blk * 2 + halfsel
            order = np.argsort(key, kind="stable")
            key_s = key[order]
            cnt = np.bincount(key_s, minlength=NBLK * 2)
            assert cnt.max() <= NEH, f"bucket overflow: {cnt.max()} > {NEH}"
            starts = np.zeros(NBLK * 2, dtype=np.int64)
            starts[1:] = np.cumsum(cnt)[:-1]
            pos = np.arange(len(key_s)) - starts[key_s]
            slot = key_s * NEH + pos
            pad_idx = -1 if cfg.get("NEG", 1) else 0
            idxflat = np.full(NBLK * 2 * NEH, pad_idx, dtype=np.int16)
            dstflat = np.full(NBLK * 2 * NEH, -1.0, dtype=np.float32)
            sv = srow[order] - halfsel[order] * HALF
            for eb in np.nonzero(cnt == 0)[0]:
                idxflat[eb * NEH] = 0
            idxflat[slot] = sv.astype(np.int16)
            dstflat[slot] = dloc[order].astype(np.float32)
            idxw16 = (idxflat.reshape(NBLK * 2, NSUB * 8, 16)
                      .transpose(2, 0, 1).reshape(16, -1))
            idxw = np.tile(idxw16, (8, 1))  # replicated across the 8 Q7 cores
            dstw = (dstflat.reshape(NBLK * 2, NSUB, P).transpose(2, 0, 1)
                    .reshape(P, NBLK * 2 * NSUB)).astype(NP_BF)
            cc = (np.maximum(cnt, 1) if cfg.get("NEG", 1)
                  else np.full_like(cnt, NEH))
            packs.append((idxw, np.ascontiguousarray(dstw),
                          cc.reshape(1, -1).astype(np.int32)))

        in_maps.append({
            "xT": xT, "w1e": w1e, "w2e": w2e,
            "b1r": b1.reshape(1, HD).astype(np.float32),
            "b2r": b2.reshape(1, 2).astype(np.float32),
            "idx16": packs[0][0], "dstf": packs[0][1], "cnts": packs[0][2],
            "idx16b": packs[1][0], "dstfb": packs[1][1], "cnts2": packs[1][2],
        })
    return in_maps


_NC_CACHE = {}


def _get_nc():
    if "nc" not in _NC_CACHE:
        _NC_CACHE["nc"] = build_nc(FULL_CFG)
    return _NC_CACHE["nc"]


def kernel(**inputs):
    from concourse.bass_utils import run_bass_kernel_spmd

    nc = _get_nc()
    in_maps = host_prep(inputs, FULL_CFG)
    res = run_bass_kernel_spmd(nc, in_maps, core_ids=list(range(FULL_CFG["NCORES"])))
    out = np.concatenate([r["out"] for r in res.results])[:FULL_CFG["N"]]
    return np.ascontiguousarray(out.astype(np.float32))


# revision 26
# speedup vs baseline: 1.4884x; 1.1876x over previous
"""Self-contained Trainium2 Bass kernel for the 2-layer GAT problem.

Accepts FULL inputs, shards destination-node ranges across 8 NeuronCores
internally, and returns the FULL [50000, 2] float32 output.

Structure (per core):
  P1: replicated node transform x@[W1|W1@A1s|W1@A1d] -> DRAM table rows
      [h(256) | asrc(8) | adst(8)] bf16, padded to 384-col rows (768B gather
      elems). adst captured into SBUF on the fly.
  P2: per (dst-block, src-half) bucket of <=1280 edges: dma_gather source
      rows (skip -1 pads), batched one-hot S build, PE-transpose -> ST,
      per-subtile adst broadcast via tiny matmuls, leakyrelu-softmax weights
      via exp/exp(0.2x)/max, aggregation matmuls with the weight column
      appended to the rhs (denominator accumulates in PSUM cols 256:264).
      Software-pipelined with a 1-bucket lookahead.
  AllGather of the layer-2 node table in 7 chunks (overlapped under P2),
      chunk-major table2 layout so each chunk is a contiguous AG output.
  P3: same machinery on the 4-wide layer-2 rows (256B gather elems).
"""
import numpy as np

import concourse.bacc as bacc
import concourse.mybir as mybir
import concourse.tile as tile
from concourse.masks import make_identity

F32 = mybir.dt.float32
BF = mybir.dt.bfloat16
I16 = mybir.dt.int16
NP_BF = mybir.dt.np(BF)

H = 8       # heads
C = 32      # per-head channels
HD = H * C  # 256
FIN = 128
ELEM = 384   # table row elems (768B); cols 0:272 used
ELEM2 = 128  # table2 row elems (256B); cols 0:4 used
P = 128

import os as _os

FULL_CFG = dict(
    N=50000, NPAD=50176, PER=6272, NBLK=49, HALF=25088, NSUB=10, NCORES=8,
    XCHUNK=1024,
    NAG=int(_os.environ.get("GAT_NAG", "7")),
    NEG=int(_os.environ.get("GAT_NEG", "1")),
    NSWQ=int(_os.environ.get("GAT_NSWQ", "1")),
    SP=int(_os.environ.get("GAT_SP", "0")),
    SP3=int(_os.environ.get("GAT_SP3", "0")),
)


def build_nc(cfg):
    NPAD, PER, NBLK, HALF, NSUB = (
        cfg["NPAD"], cfg["PER"], cfg["NBLK"], cfg["HALF"], cfg["NSUB"])
    NCORES = cfg["NCORES"]
    XCHUNK = cfg["XCHUNK"]
    NAG = cfg["NAG"]
    NEH = NSUB * P                # idxs per (block, half) gather
    IDXW = NEH // 16              # idx cols per bucket
    NTILE = NPAD // P             # node tiles in P1
    NB2 = NBLK * 2                # buckets
    CBLK = NBLK // NAG            # blocks per AG chunk
    CROWS = CBLK * P              # local rows per AG chunk
    assert NPAD == NCORES * PER and PER == NBLK * P and NPAD % XCHUNK == 0
    assert HALF % P == 0 and 2 * HALF == NPAD and NBLK == NAG * CBLK

    NSWQ = cfg.get("NSWQ", 1)
    SP = bool(cfg.get("SP", 0))
    NEG = bool(cfg.get("NEG", 1))
    SP3 = bool(cfg.get("SP3", 0))
    nc = bacc.Bacc(None, target_bir_lowering=False, num_devices=NCORES,
                   num_swdge_queues=NSWQ)

    xT_d = nc.dram_tensor("xT", [FIN, NPAD], BF, kind="ExternalInput")
    w1e_d = nc.dram_tensor("w1e", [FIN, 272], BF, kind="ExternalInput")
    w2e_d = nc.dram_tensor("w2e", [P, 8], BF, kind="ExternalInput")
    b1_d = nc.dram_tensor("b1r", [1, HD], F32, kind="ExternalInput")
    b2_d = nc.dram_tensor("b2r", [1, 2], F32, kind="ExternalInput")
    idx_d = nc.dram_tensor("idx16", [P, NB2 * IDXW], I16, kind="ExternalInput")
    idx2_d = nc.dram_tensor("idx16b", [P, NB2 * IDXW], I16, kind="ExternalInput")
    dst_d = nc.dram_tensor("dstf", [P, NB2 * NSUB], BF, kind="ExternalInput")
    dst2_d = nc.dram_tensor("dstfb", [P, NB2 * NSUB], BF, kind="ExternalInput")
    cnt_d = nc.dram_tensor("cnts", [1, NB2], mybir.dt.int32, kind="ExternalInput")
    cnt2_d = nc.dram_tensor("cnts2", [1, NB2], mybir.dt.int32, kind="ExternalInput")
    out_d = nc.dram_tensor("out", [PER, 2], F32, kind="ExternalOutput")

    table = nc.dram_tensor("table", [NPAD, ELEM], BF)
    t2locs = [nc.dram_tensor(f"t2loc{k}", [CROWS, ELEM2], BF) for k in range(NAG)]
    table2 = nc.dram_tensor("table2", [NPAD, ELEM2], BF)

    with tile.TileContext(nc) as tc:
        with (
            tc.tile_pool(name="cst", bufs=1) as cst,
            tc.tile_pool(name="xp", bufs=2) as xp,
            tc.tile_pool(name="rowp", bufs=3) as rowp,
            tc.tile_pool(name="gp", bufs=3) as gp,
            tc.tile_pool(name="g2p", bufs=3) as g2p,
            tc.tile_pool(name="sp", bufs=3) as sp,
            tc.tile_pool(name="mp", bufs=3) as mp,
            tc.tile_pool(name="tailp", bufs=2) as tailp,
            tc.tile_pool(name="ps", bufs=2, space="PSUM") as ps,
        ):
            # ---- constants ----
            ident = cst.tile([P, P], BF)
            make_identity(nc, ident[:])
            iota_i = cst.tile([P, P], I16)
            nc.gpsimd.iota(iota_i[:], pattern=[[1, P]], base=0, channel_multiplier=0)
            iota_bf = cst.tile([P, P], BF)
            nc.vector.tensor_copy(iota_bf[:], iota_i[:])
            iota_rep = cst.tile([P, NSUB, P], BF)
            nc.vector.tensor_copy(
                iota_rep[:], iota_bf[:, None, :].to_broadcast([P, NSUB, P]))
            onesk = cst.tile([1, P], F32)
            nc.vector.memset(onesk[:], 1.0)

            w1e_sb = cst.tile([FIN, 272], BF)
            nc.sync.dma_start(out=w1e_sb[:], in_=w1e_d[:])
            w2e_sb = cst.tile([P, 2, 4], BF)
            nc.sync.dma_start(out=w2e_sb[:], in_=w2e_d[:].rearrange("p (k n) -> p k n", k=2))
            idx_sb = cst.tile([P, NB2 * IDXW], I16)
            nc.sync.dma_start(out=idx_sb[:], in_=idx_d[:])
            idx2_sb = cst.tile([P, NB2 * IDXW], I16)
            nc.sync.dma_start(out=idx2_sb[:], in_=idx2_d[:])
            dst_sb = cst.tile([P, NB2 * NSUB], BF)
            nc.sync.dma_start(out=dst_sb[:], in_=dst_d[:])
            dst2_sb = cst.tile([P, NB2 * NSUB], BF)
            nc.sync.dma_start(out=dst2_sb[:], in_=dst2_d[:])
            cnt_sb = cst.tile([1, NB2], mybir.dt.int32)
            nc.sync.dma_start(out=cnt_sb[:], in_=cnt_d[:])
            cnt2_sb = cst.tile([1, NB2], mybir.dt.int32)
            nc.sync.dma_start(out=cnt2_sb[:], in_=cnt2_d[:])

            # bias broadcast rows -> [P, HD], [P, 2]
            b1r = cst.tile([1, HD], F32)
            nc.sync.dma_start(out=b1r[:], in_=b1_d[:])
            b2r = cst.tile([1, 2], F32)
            nc.sync.dma_start(out=b2r[:], in_=b2_d[:])
            bps = ps.tile([P, HD], F32, space="PSUM", tag="aexp", bufs=3)
            nc.tensor.matmul(out=bps[:], lhsT=onesk[:], rhs=b1r[:], start=True, stop=True)
            b1bc = cst.tile([P, HD], F32)
            nc.scalar.copy(b1bc[:], bps[:])
            bps2 = ps.tile([P, 2], F32, space="PSUM", tag="aexp", bufs=3)
            nc.tensor.matmul(out=bps2[:], lhsT=onesk[:], rhs=b2r[:], start=True, stop=True)
            b2bc = cst.tile([P, 2], F32)
            nc.scalar.copy(b2bc[:], bps2[:])

            adst_sb = cst.tile([P, NBLK, H], BF)
            adst2_sb = cst.tile([P, NBLK], BF)
            outstage = cst.tile([P, NBLK, 2], F32)

            # prime gather buffers so skipped (-1) slots read finite data
            for _ in range(3):
                gz = gp.tile([P, NSUB, ELEM], BF, tag="g")
                nc.vector.memset(gz[:], 0.0)
                g2z = g2p.tile([P, NSUB, ELEM2], BF, tag="g2")
                nc.vector.memset(g2z[:], 0.0)

            # ---- P1: node features -> table (replicated over all nodes) ----
            for ch in range(NPAD // XCHUNK):
                xc = xp.tile([FIN, XCHUNK], BF, tag="xc")
                nc.sync.dma_start(out=xc[:], in_=xT_d[:, ch * XCHUNK:(ch + 1) * XCHUNK])
                for j in range(XCHUNK // P):
                    nt = ch * (XCHUNK // P) + j
                    ph = ps.tile([P, 272], F32, space="PSUM", tag="accum")
                    nc.tensor.matmul(out=ph[:], lhsT=xc[:, j * P:(j + 1) * P],
                                     rhs=w1e_sb[:], start=True, stop=True)
                    row = rowp.tile([P, 272], BF, tag="row")
                    if nt % 2 == 0:
                        nc.scalar.copy(row[:], ph[:])
                    else:
                        nc.vector.tensor_copy(row[:], ph[:])
                    nc.sync.dma_start(out=table[nt * P:(nt + 1) * P, 0:272], in_=row[:])

            # ---- adst slice for own dst range (pid ladder) ----
            pid = nc.sync.partition_id()
            for c in range(NCORES):
                with tc.If(pid == c):
                    nc.sync.dma_start(
                        out=adst_sb[:],
                        in_=table[c * PER:(c + 1) * PER, 264:272]
                            .rearrange("(b p) h -> p b h", p=P))

            # =========== P2: layer-1 message passing (pipelined) ===========
            state = {}
            stateE = {}
            pblks = {}

            def prepE2(k):
                b = k // 2
                half = k % 2
                g = gp.tile([P, NSUB, ELEM], BF, tag="g")
                if NEG:
                    creg = nc.gpsimd.alloc_register()
                    nc.gpsimd.reg_load(creg, cnt_sb[:, k:k + 1])
                    nreg = creg
                else:
                    nreg = NEH
                nc.gpsimd.dma_gather(
                    out_ap=g[:],
                    in_ap=(table[0:HALF, :] if half == 0 else table[HALF:NPAD, :]),
                    idxs_ap=idx_sb[:, k * IDXW:(k + 1) * IDXW],
                    num_idxs=NEH, num_idxs_reg=nreg, elem_size=ELEM,
                    single_packet=SP, queue_num=k % NSWQ)
                S = sp.tile([P, NSUB, P], BF, tag="S")
                nc.vector.tensor_tensor(
                    out=S[:], in0=iota_rep[:],
                    in1=dst_sb[:, k * NSUB:(k + 1) * NSUB][:, :, None]
                        .to_broadcast([P, NSUB, P]),
                    op=mybir.AluOpType.is_equal)
                ST = sp.tile([P, NSUB, P], BF, tag="ST", bufs=3)
                for grp in range(2):
                    tps = ps.tile([P, 5, P], BF, space="PSUM", tag="tps")
                    for tt in range(5):
                        t = grp * 5 + tt
                        nc.tensor.transpose(tps[:, tt, :], S[:, t, :], ident[:])
                    nc.scalar.copy(ST[:, grp * 5:(grp + 1) * 5, :], tps[:])
                aexp = ps.tile([P, NSUB, H], F32, space="PSUM", tag="aexp", bufs=3)
                for t in range(NSUB):
                    nc.tensor.matmul(out=aexp[:, t, :], lhsT=ST[:, t, :],
                                     rhs=adst_sb[:, b, :], start=True, stop=True)
                stateE[k] = (g, S, aexp)

            def prepL2(k):
                g, S, aexp = stateE.pop(k)
                logits = mp.tile([P, NSUB, H], F32, tag="logits")
                nc.vector.tensor_tensor(out=logits[:], in0=g[:, :, 256:264],
                                        in1=aexp[:], op=mybir.AluOpType.add)
                e1 = mp.tile([P, NSUB, H], F32, tag="e1")
                nc.scalar.activation(e1[:], logits[:], mybir.ActivationFunctionType.Exp)
                e2 = mp.tile([P, NSUB, H], F32, tag="e2")
                nc.scalar.activation(e2[:], logits[:], mybir.ActivationFunctionType.Exp,
                                     scale=0.2)
                msg = mp.tile([P, NSUB, 264], BF, tag="msg")
                wtv = msg[:, :, 256:264]
                nc.vector.tensor_tensor(out=wtv, in0=e1[:], in1=e2[:],
                                        op=mybir.AluOpType.max)
                nc.vector.tensor_tensor(
                    out=msg[:, :, 0:256].rearrange("p t (h c) -> p t h c", h=H),
                    in0=g[:, :, 0:256].rearrange("p t (h c) -> p t h c", h=H),
                    in1=wtv[:, :, :, None].to_broadcast([P, NSUB, H, C]),
                    op=mybir.AluOpType.mult)
                state[k] = (S, msg)

            def agg2(k):
                b, half = k // 2, k % 2
                S, msg = state.pop(k)
                if half == 0:
                    pblks[b] = ps.tile([P, 264], F32, space="PSUM", tag="accum", name="pblk")
                pblk = pblks[b]
                for t in range(NSUB):
                    nc.tensor.matmul(out=pblk[:], lhsT=S[:, t, :],
                                     rhs=msg[:, t, :],
                                     start=(half == 0 and t == 0),
                                     stop=(half == 1 and t == NSUB - 1))

            def tail1(b):
                pb = pblks.pop(b)
                srec = tailp.tile([P, H], F32, tag="srec")
                nc.vector.tensor_scalar(
                    out=srec[:], in0=pb[:, 256:264], scalar1=1e-16, scalar2=None,
                    op0=mybir.AluOpType.add)
                rec = tailp.tile([P, H], F32, tag="rec")
                nc.vector.reciprocal(rec[:], srec[:])
                out1 = tailp.tile([P, HD], F32, tag="out1")
                nc.vector.tensor_tensor(
                    out=out1[:].rearrange("p (h c) -> p h c", h=H),
                    in0=pb[:, 0:256].rearrange("p (h c) -> p h c", h=H),
                    in1=rec[:, :, None].to_broadcast([P, H, C]),
                    op=mybir.AluOpType.mult)
                v = tailp.tile([P, HD], F32, tag="v")
                nc.vector.tensor_tensor(out=v[:], in0=out1[:], in1=b1bc[:],
                                        op=mybir.AluOpType.add)
                ev = tailp.tile([P, HD], F32, tag="ev")
                nc.scalar.activation(ev[:], v[:], mybir.ActivationFunctionType.Exp)
                em = tailp.tile([P, HD], F32, tag="em")
                nc.vector.tensor_scalar(out=em[:], in0=ev[:], scalar1=1.0, scalar2=0.0,
                                        op0=mybir.AluOpType.subtract,
                                        op1=mybir.AluOpType.min)
                pp = tailp.tile([P, HD], F32, tag="pp")
                nc.vector.tensor_scalar(out=pp[:], in0=v[:], scalar1=0.0, scalar2=None,
                                        op0=mybir.AluOpType.max)
                elu = tailp.tile([P, HD], BF, tag="elu")
                nc.vector.tensor_tensor(out=elu[:], in0=em[:], in1=pp[:],
                                        op=mybir.AluOpType.add)
                eT_ps = ps.tile([P, 2, P], BF, space="PSUM", tag="tps")
                for kk in range(2):
                    nc.tensor.transpose(eT_ps[:, kk, :], elu[:, kk * P:(kk + 1) * P],
                                        ident[:])
                eT_sb = tailp.tile([P, 2, P], BF, tag="eT")
                nc.scalar.copy(eT_sb[:], eT_ps[:])
                ph2 = ps.tile([P, 4], F32, space="PSUM", tag="aexp", bufs=3)
                for kk in range(2):
                    nc.tensor.matmul(out=ph2[:], lhsT=eT_sb[:, kk, :],
                                     rhs=w2e_sb[:, kk, :],
                                     start=(kk == 0), stop=(kk == 1))
                t2row = tailp.tile([P, 4], BF, tag="t2row")
                nc.vector.tensor_copy(t2row[:], ph2[:, 0:4])
                nc.vector.tensor_copy(adst2_sb[:, b:b + 1], ph2[:, 3:4])
                kag = b // CBLK
                rb = (b % CBLK) * P
                nc.sync.dma_start(out=t2locs[kag][rb:rb + P, 0:4], in_=t2row[:])

            for k in range(NB2 + 2):
                if k >= 2:
                    agg2(k - 2)
                    if k % 2 == 1:
                        b = (k - 2) // 2
                        tail1(b)
                        if (b + 1) % CBLK == 0:
                            kag = b // CBLK
                            nc.gpsimd.collective_compute(
                                "AllGather", mybir.AluOpType.bypass,
                                replica_groups=[list(range(NCORES))],
                                ins=[t2locs[kag][:]],
                                outs=[table2[kag * NCORES * CROWS:
                                             (kag + 1) * NCORES * CROWS, :]])
                if k < NB2:
                    prepE2(k)
                if 1 <= k <= NB2:
                    prepL2(k - 1)

            # =========== P3: layer-2 message passing (pipelined) ===========
            def prepE3(k):
                b = k // 2
                half = k % 2
                g2 = g2p.tile([P, NSUB, ELEM2], BF, tag="g2")
                if NEG:
                    creg = nc.gpsimd.alloc_register()
                    nc.gpsimd.reg_load(creg, cnt2_sb[:, k:k + 1])
                    nreg = creg
                else:
                    nreg = NEH
                nc.gpsimd.dma_gather(
                    out_ap=g2[:],
                    in_ap=(table2[0:HALF, :] if half == 0 else table2[HALF:NPAD, :]),
                    idxs_ap=idx2_sb[:, k * IDXW:(k + 1) * IDXW],
                    num_idxs=NEH, num_idxs_reg=nreg, elem_size=ELEM2,
                    single_packet=SP3, queue_num=k % NSWQ)
                S = sp.tile([P, NSUB, P], BF, tag="S")
                nc.vector.tensor_tensor(
                    out=S[:], in0=iota_rep[:],
                    in1=dst2_sb[:, k * NSUB:(k + 1) * NSUB][:, :, None]
                        .to_broadcast([P, NSUB, P]),
                    op=mybir.AluOpType.is_equal)
                ST = sp.tile([P, NSUB, P], BF, tag="ST", bufs=3)
                for grp in range(2):
                    tps = ps.tile([P, 5, P], BF, space="PSUM", tag="tps")
                    for tt in range(5):
                        t = grp * 5 + tt
                        nc.tensor.transpose(tps[:, tt, :], S[:, t, :], ident[:])
                    nc.scalar.copy(ST[:, grp * 5:(grp + 1) * 5, :], tps[:])
                a2e = ps.tile([P, NSUB], F32, space="PSUM", tag="aexp", bufs=3)
                for t in range(NSUB):
                    nc.tensor.matmul(out=a2e[:, t:t + 1], lhsT=ST[:, t, :],
                                     rhs=adst2_sb[:, b:b + 1], start=True, stop=True)
                stateE[k] = (g2, S, a2e)

            def prepL3(k):
                g2, S, a2e = stateE.pop(k)
                lg2 = mp.tile([P, NSUB], F32, tag="lg2")
                nc.vector.tensor_tensor(
                    out=lg2[:], in0=g2[:, :, 2:3].rearrange("p t x -> p (t x)"),
                    in1=a2e[:], op=mybir.AluOpType.add)
                f1 = mp.tile([P, NSUB], F32, tag="f1")
                nc.scalar.activation(f1[:], lg2[:], mybir.ActivationFunctionType.Exp)
                f2 = mp.tile([P, NSUB], F32, tag="f2")
                nc.scalar.activation(f2[:], lg2[:], mybir.ActivationFunctionType.Exp,
                                     scale=0.2)
                msg2 = mp.tile([P, NSUB, 3], BF, tag="msg2")
                w2tv = msg2[:, :, 2:3].rearrange("p t x -> p (t x)")
                nc.vector.tensor_tensor(out=w2tv, in0=f1[:], in1=f2[:],
                                        op=mybir.AluOpType.max)
                nc.vector.tensor_tensor(
                    out=msg2[:, :, 0:2], in0=g2[:, :, 0:2],
                    in1=msg2[:, :, 2:3].to_broadcast([P, NSUB, 2]),
                    op=mybir.AluOpType.mult)
                state[k] = (S, msg2)

            def agg3(k):
                b, half = k // 2, k % 2
                S, msg2 = state.pop(k)
                if half == 0:
                    pblks[b] = ps.tile([P, 3], F32, space="PSUM", tag="accum", name="p2s")
                p2s = pblks[b]
                for t in range(NSUB):
                    nc.tensor.matmul(out=p2s[:], lhsT=S[:, t, :],
                                     rhs=msg2[:, t, :],
                                     start=(half == 0 and t == 0),
                                     stop=(half == 1 and t == NSUB - 1))

            def tail3(b):
                p2s = pblks.pop(b)
                s2r = tailp.tile([P, 1], F32, tag="s2r")
                nc.vector.tensor_scalar(out=s2r[:], in0=p2s[:, 2:3], scalar1=1e-16,
                                        scalar2=None, op0=mybir.AluOpType.add)
                rec2 = tailp.tile([P, 1], F32, tag="rec2")
                nc.vector.reciprocal(rec2[:], s2r[:])
                o2 = tailp.tile([P, 2], F32, tag="o2")
                nc.vector.tensor_tensor(out=o2[:], in0=p2s[:, 0:2],
                                        in1=rec2[:].to_broadcast([P, 2]),
                                        op=mybir.AluOpType.mult)
                nc.vector.tensor_tensor(out=outstage[:, b, :], in0=o2[:], in1=b2bc[:],
                                        op=mybir.AluOpType.add)

            for k in range(NB2 + 2):
                if k >= 2:
                    agg3(k - 2)
                    if k % 2 == 1:
                        tail3((k - 2) // 2)
                if k < NB2:
                    prepE3(k)
                if 1 <= k <= NB2:
                    prepL3(k - 1)

            nc.sync.dma_start(
                out=out_d[:].rearrange("(b p) c -> p b c", p=P), in_=outstage[:])

    nc.compile()
    return nc


def host_prep(inputs, cfg):
    """Build per-core input maps from full inputs."""
    N, NPAD, PER, NBLK, HALF, NSUB, NCORES, NAG = (
        cfg["N"], cfg["NPAD"], cfg["PER"], cfg["NBLK"], cfg["HALF"],
        cfg["NSUB"], cfg["NCORES"], cfg["NAG"])
    NEH = NSUB * P
    IDXW = NEH // 16
    CBLK = NBLK // NAG

    x = np.asarray(inputs["x"], dtype=np.float32)
    ei = np.asarray(inputs["edge_index"], dtype=np.int64)
    W1 = np.asarray(inputs["W1"], dtype=np.float64)
    a1s = np.asarray(inputs["a1_src"], dtype=np.float64)
    a1d = np.asarray(inputs["a1_dst"], dtype=np.float64)
    b1 = np.asarray(inputs["b1"], dtype=np.float32)
    W2 = np.asarray(inputs["W2"], dtype=np.float64)
    a2s = np.asarray(inputs["a2_src"], dtype=np.float64)
    a2d = np.asarray(inputs["a2_dst"], dtype=np.float64)
    b2 = np.asarray(inputs["b2"], dtype=np.float32)

    xT = np.zeros((FIN, NPAD), dtype=np.float32)
    xT[:, :N] = x.T
    xT = xT.astype(NP_BF)

    A1s = np.zeros((HD, H))
    A1d = np.zeros((HD, H))
    for hd in range(H):
        A1s[hd * C:(hd + 1) * C, hd] = a1s[hd]
        A1d[hd * C:(hd + 1) * C, hd] = a1d[hd]
    w1e = np.concatenate([W1, W1 @ A1s, W1 @ A1d], axis=1).astype(NP_BF)  # [128,272]

    w2cols = np.concatenate([W2, W2 @ a2s[0][:, None], W2 @ a2d[0][:, None]],
                            axis=1)  # [HD, 4]
    w2e = w2cols.reshape(2, P, 4).transpose(1, 0, 2).reshape(P, 8).astype(NP_BF)

    loops = np.arange(N, dtype=np.int64)
    src = np.concatenate([ei[0], loops])
    dst = np.concatenate([ei[1], loops])

    # chunk-major table2 row index for each node
    nodes = np.arange(NPAD, dtype=np.int64)
    n_c, n_r = nodes // PER, nodes % PER
    n_b, n_j = n_r >> 7, n_r & 127
    n_k = n_b // CBLK
    row2_of = (n_k * (NCORES * CBLK * P) + n_c * (CBLK * P)
               + (n_b - n_k * CBLK) * P + n_j)

    in_maps = []
    for c in range(NCORES):
        lo_n, hi_n = c * PER, (c + 1) * PER
        m = (dst >= lo_n) & (dst < hi_n)
        s_c = src[m]
        d_c = dst[m] - lo_n
        blk = d_c >> 7
        dloc = d_c & 127

        packs = []
        for srow in (s_c, row2_of[s_c]):
            halfsel = (srow >= HALF).astype(np.int64)
            key = blk * 2 + halfsel
            order = np.argsort(key, kind="stable")
            key_s = key[order]
            cnt = np.bincount(key_s, minlength=NBLK * 2)
            assert cnt.max() <= NEH, f"bucket overflow: {cnt.max()} > {NEH}"
            starts = np.zeros(NBLK * 2, dtype=np.int64)
            starts[1:] = np.cumsum(cnt)[:-1]
            pos = np.arange(len(key_s)) - starts[key_s]
            slot = key_s * NEH + pos
            pad_idx = -1 if cfg.get("NEG", 1) else 0
            idxflat = np.full(NBLK * 2 * NEH, pad_idx, dtype=np.int16)
            dstflat = np.full(NBLK * 2 * NEH, -1.0, dtype=np.float32)
            sv = srow[order] - halfsel[order] * HALF
            for eb in np.nonzero(cnt == 0)[0]:
                idxflat[eb * NEH] = 0
            idxflat[slot] = sv.astype(np.int16)
            dstflat[slot] = dloc[order].astype(np.float32)
            idxw16 = (idxflat.reshape(NBLK * 2, NSUB * 8, 16)
                      .transpose(2, 0, 1).reshape(16, -1))
            idxw = np.tile(idxw16, (8, 1))  # replicated across the 8 Q7 cores
            dstw = (dstflat.reshape(NBLK * 2, NSUB, P).transpose(2, 0, 1)
                    .reshape(P, NBLK * 2 * NSUB)).astype(NP_BF)
            cc = (np.maximum(cnt, 1) if cfg.get("NEG", 1)
                  else np.full_like(cnt, NEH))
            packs.append((idxw, np.ascontiguousarray(dstw),
                          cc.reshape(1, -1).astype(np.int32)))

        in_maps.append({
            "xT": xT, "w1e": w1e, "w2e": w2e,
            "b1r": b1.reshape(1, HD).astype(np.float32),
            "b2r": b2.reshape(1, 2).astype(np.float32),
            "idx16": packs[0][0], "dstf": packs[0][1], "cnts": packs[0][2],
            "idx16b": packs[1][0], "dstfb": packs[1][1], "cnts2": packs[1][2],
        })
    return in_maps


_NC_CACHE = {}


def _get_nc():
    if "nc" not in _NC_CACHE:
        _NC_CACHE["nc"] = build_nc(FULL_CFG)
    return _NC_CACHE["nc"]


def kernel(**inputs):
    from concourse.bass_utils import run_bass_kernel_spmd

    nc = _get_nc()
    in_maps = host_prep(inputs, FULL_CFG)
    res = run_bass_kernel_spmd(nc, in_maps, core_ids=list(range(FULL_CFG["NCORES"])))
    out = np.concatenate([r["out"] for r in res.results])[:FULL_CFG["N"]]
    return np.ascontiguousarray(out.astype(np.float32))


# revision 27
# speedup vs baseline: 1.7038x; 1.1447x over previous
"""Self-contained Trainium2 Bass kernel for the 2-layer GAT problem.

Accepts FULL inputs, shards destination-node ranges across 8 NeuronCores
internally, and returns the FULL [50000, 2] float32 output.

Structure (per core):
  P1: replicated node transform x@[W1|W1@A1s|W1@A1d] -> DRAM table rows
      [h(256) | asrc(8) | adst(8)] bf16, padded to 384-col rows (768B gather
      elems). adst captured into SBUF on the fly.
  P2: per (dst-block, src-half) bucket of <=1280 edges: dma_gather source
      rows (skip -1 pads), batched one-hot S build, PE-transpose -> ST,
      per-subtile adst broadcast via tiny matmuls, leakyrelu-softmax weights
      via exp/exp(0.2x)/max, aggregation matmuls with the weight column
      appended to the rhs (denominator accumulates in PSUM cols 256:264).
      Software-pipelined with a 1-bucket lookahead.
  AllGather of the layer-2 node table in 7 chunks (overlapped under P2),
      chunk-major table2 layout so each chunk is a contiguous AG output.
  P3: same machinery on the 4-wide layer-2 rows (256B gather elems).
"""
import numpy as np

import concourse.bacc as bacc
import concourse.mybir as mybir
import concourse.tile as tile
from concourse.masks import make_identity

F32 = mybir.dt.float32
BF = mybir.dt.bfloat16
I16 = mybir.dt.int16
NP_BF = mybir.dt.np(BF)

H = 8       # heads
C = 32      # per-head channels
HD = H * C  # 256
FIN = 128
ELEM = 384   # table row elems (768B); cols 0:272 used
ELEM2 = 128  # table2 row elems (256B); cols 0:4 used
P = 128

import os as _os

FULL_CFG = dict(
    N=50000, NPAD=50176, PER=6272, NBLK=49, HALF=25088, NSUB=10, NCORES=8,
    XCHUNK=1024,
    NAG=int(_os.environ.get("GAT_NAG", "7")),
    NEG=int(_os.environ.get("GAT_NEG", "1")),
    NSWQ=int(_os.environ.get("GAT_NSWQ", "1")),
    SP=int(_os.environ.get("GAT_SP", "0")),
    SP3=int(_os.environ.get("GAT_SP3", "0")),
)


def build_nc(cfg):
    NPAD, PER, NBLK, HALF, NSUB = (
        cfg["NPAD"], cfg["PER"], cfg["NBLK"], cfg["HALF"], cfg["NSUB"])
    NCORES = cfg["NCORES"]
    XCHUNK = cfg["XCHUNK"]
    NAG = cfg["NAG"]
    NEH = NSUB * P                # idxs per (block, half) gather
    IDXW = NEH // 16              # idx cols per bucket
    NTILE = NPAD // P             # node tiles in P1
    NB2 = NBLK * 2                # buckets
    CBLK = NBLK // NAG            # blocks per AG chunk
    CROWS = CBLK * P              # local rows per AG chunk
    assert NPAD == NCORES * PER and PER == NBLK * P and NPAD % XCHUNK == 0
    assert HALF % P == 0 and 2 * HALF == NPAD and NBLK == NAG * CBLK

    NSWQ = cfg.get("NSWQ", 1)
    SP = bool(cfg.get("SP", 0))
    NEG = bool(cfg.get("NEG", 1))
    SP3 = bool(cfg.get("SP3", 0))
    nc = bacc.Bacc(None, target_bir_lowering=False, num_devices=NCORES,
                   num_swdge_queues=NSWQ)

    xT_d = nc.dram_tensor("xT", [FIN, NPAD], BF, kind="ExternalInput")
    w1e_d = nc.dram_tensor("w1e", [FIN, 272], BF, kind="ExternalInput")
    w2e_d = nc.dram_tensor("w2e", [P, 8], BF, kind="ExternalInput")
    b1_d = nc.dram_tensor("b1r", [1, HD], F32, kind="ExternalInput")
    b2_d = nc.dram_tensor("b2r", [1, 2], F32, kind="ExternalInput")
    idx_d = nc.dram_tensor("idx16", [P, NB2 * IDXW], I16, kind="ExternalInput")
    idx2_d = nc.dram_tensor("idx16b", [P, NB2 * IDXW], I16, kind="ExternalInput")
    dst_d = nc.dram_tensor("dstf", [P, NB2 * NSUB], BF, kind="ExternalInput")
    dst2_d = nc.dram_tensor("dstfb", [P, NB2 * NSUB], BF, kind="ExternalInput")
    cnt_d = nc.dram_tensor("cnts", [1, NB2], mybir.dt.int32, kind="ExternalInput")
    cnt2_d = nc.dram_tensor("cnts2", [1, NB2], mybir.dt.int32, kind="ExternalInput")
    out_d = nc.dram_tensor("out", [PER, 2], F32, kind="ExternalOutput")

    table = nc.dram_tensor("table", [NPAD, ELEM], BF)
    t2locs = [nc.dram_tensor(f"t2loc{k}", [CROWS, ELEM2], BF) for k in range(NAG)]
    table2 = nc.dram_tensor("table2", [NPAD, ELEM2], BF)

    with tile.TileContext(nc) as tc:
        with (
            tc.tile_pool(name="cst", bufs=1) as cst,
            tc.tile_pool(name="xp", bufs=2) as xp,
            tc.tile_pool(name="rowp", bufs=3) as rowp,
            tc.tile_pool(name="gp", bufs=3) as gp,
            tc.tile_pool(name="g2p", bufs=3) as g2p,
            tc.tile_pool(name="sp", bufs=3) as sp,
            tc.tile_pool(name="mp", bufs=3) as mp,
            tc.tile_pool(name="tailp", bufs=2) as tailp,
            tc.tile_pool(name="ps", bufs=2, space="PSUM") as ps,
        ):
            # ---- constants ----
            ident = cst.tile([P, P], BF)
            make_identity(nc, ident[:])
            iota_i = cst.tile([P, P], I16)
            nc.gpsimd.iota(iota_i[:], pattern=[[1, P]], base=0, channel_multiplier=0)
            iota_bf = cst.tile([P, P], BF)
            nc.vector.tensor_copy(iota_bf[:], iota_i[:])
            iota_rep = cst.tile([P, NSUB, P], BF)
            nc.vector.tensor_copy(
                iota_rep[:], iota_bf[:, None, :].to_broadcast([P, NSUB, P]))
            onesk = cst.tile([1, P], F32)
            nc.vector.memset(onesk[:], 1.0)

            w1e_sb = cst.tile([FIN, 272], BF)
            nc.sync.dma_start(out=w1e_sb[:], in_=w1e_d[:])
            w2e_sb = cst.tile([P, 2, 4], BF)
            nc.sync.dma_start(out=w2e_sb[:], in_=w2e_d[:].rearrange("p (k n) -> p k n", k=2))
            idx_sb = cst.tile([P, NB2 * IDXW], I16)
            nc.sync.dma_start(out=idx_sb[:], in_=idx_d[:])
            idx2_sb = cst.tile([P, NB2 * IDXW], I16)
            nc.sync.dma_start(out=idx2_sb[:], in_=idx2_d[:])
            dst_sb = cst.tile([P, NB2 * NSUB], BF)
            nc.sync.dma_start(out=dst_sb[:], in_=dst_d[:])
            dst2_sb = cst.tile([P, NB2 * NSUB], BF)
            nc.sync.dma_start(out=dst2_sb[:], in_=dst2_d[:])
            cnt_sb = cst.tile([1, NB2], mybir.dt.int32)
            nc.sync.dma_start(out=cnt_sb[:], in_=cnt_d[:])
            cnt2_sb = cst.tile([1, NB2], mybir.dt.int32)
            nc.sync.dma_start(out=cnt2_sb[:], in_=cnt2_d[:])

            # bias broadcast rows -> [P, HD], [P, 2]
            b1r = cst.tile([1, HD], F32)
            nc.sync.dma_start(out=b1r[:], in_=b1_d[:])
            b2r = cst.tile([1, 2], F32)
            nc.sync.dma_start(out=b2r[:], in_=b2_d[:])
            bps = ps.tile([P, HD], F32, space="PSUM", tag="aexp", bufs=3)
            nc.tensor.matmul(out=bps[:], lhsT=onesk[:], rhs=b1r[:], start=True, stop=True)
            b1bc = cst.tile([P, HD], F32)
            nc.scalar.copy(b1bc[:], bps[:])
            bps2 = ps.tile([P, 2], F32, space="PSUM", tag="aexp", bufs=3)
            nc.tensor.matmul(out=bps2[:], lhsT=onesk[:], rhs=b2r[:], start=True, stop=True)
            b2bc = cst.tile([P, 2], F32)
            nc.scalar.copy(b2bc[:], bps2[:])

            adst_sb = cst.tile([P, NBLK, H], BF)
            adst2_sb = cst.tile([P, NBLK], BF)
            outstage = cst.tile([P, NBLK, 2], F32)

            # prime gather buffers so skipped (-1) slots read finite data
            for _ in range(3):
                gz = gp.tile([P, NSUB, ELEM], BF, tag="g")
                nc.vector.memset(gz[:], 0.0)
                g2z = g2p.tile([P, NSUB, ELEM2], BF, tag="g2")
                nc.vector.memset(g2z[:], 0.0)

            # ---- P1: node features -> table (replicated over all nodes) ----
            for ch in range(NPAD // XCHUNK):
                xc = xp.tile([FIN, XCHUNK], BF, tag="xc")
                nc.sync.dma_start(out=xc[:], in_=xT_d[:, ch * XCHUNK:(ch + 1) * XCHUNK])
                for j in range(0, XCHUNK // P, 2):
                    nt = ch * (XCHUNK // P) + j
                    row = rowp.tile([P, 2, 272], BF, tag="row")
                    for q in range(2):
                        ph = ps.tile([P, 272], F32, space="PSUM", tag="accum")
                        nc.tensor.matmul(out=ph[:], lhsT=xc[:, (j + q) * P:(j + q + 1) * P],
                                         rhs=w1e_sb[:], start=True, stop=True)
                        if q == 0:
                            nc.scalar.copy(row[:, q, :], ph[:])
                        else:
                            nc.vector.tensor_copy(row[:, q, :], ph[:])
                    nc.sync.dma_start(
                        out=table[nt * P:(nt + 2) * P, 0:272]
                            .rearrange("(q p) c -> p q c", q=2),
                        in_=row[:])

            # ---- adst slice for own dst range (pid ladder) ----
            pid = nc.sync.partition_id()
            for c in range(NCORES):
                with tc.If(pid == c):
                    nc.sync.dma_start(
                        out=adst_sb[:],
                        in_=table[c * PER:(c + 1) * PER, 264:272]
                            .rearrange("(b p) h -> p b h", p=P))

            # =========== P2: layer-1 message passing (pipelined) ===========
            state = {}
            stateE = {}
            pblks = {}

            def prepE2(k):
                b = k // 2
                half = k % 2
                g = gp.tile([P, NSUB, ELEM], BF, tag="g")
                if NEG:
                    creg = nc.gpsimd.alloc_register()
                    nc.gpsimd.reg_load(creg, cnt_sb[:, k:k + 1])
                    nreg = creg
                else:
                    nreg = NEH
                nc.gpsimd.dma_gather(
                    out_ap=g[:],
                    in_ap=(table[0:HALF, :] if half == 0 else table[HALF:NPAD, :]),
                    idxs_ap=idx_sb[:, k * IDXW:(k + 1) * IDXW],
                    num_idxs=NEH, num_idxs_reg=nreg, elem_size=ELEM,
                    single_packet=SP, queue_num=k % NSWQ)
                S = sp.tile([P, NSUB, P], BF, tag="S")
                nc.vector.tensor_tensor(
                    out=S[:], in0=iota_rep[:],
                    in1=dst_sb[:, k * NSUB:(k + 1) * NSUB][:, :, None]
                        .to_broadcast([P, NSUB, P]),
                    op=mybir.AluOpType.is_equal)
                ST = sp.tile([P, NSUB, P], BF, tag="ST", bufs=3)
                for grp in range(2):
                    tps = ps.tile([P, 5, P], BF, space="PSUM", tag="tps")
                    for tt in range(5):
                        t = grp * 5 + tt
                        nc.tensor.transpose(tps[:, tt, :], S[:, t, :], ident[:])
                    nc.scalar.copy(ST[:, grp * 5:(grp + 1) * 5, :], tps[:])
                aexp = ps.tile([P, NSUB, H], F32, space="PSUM", tag="aexp", bufs=3)
                for t in range(NSUB):
                    nc.tensor.matmul(out=aexp[:, t, :], lhsT=ST[:, t, :],
                                     rhs=adst_sb[:, b, :], start=True, stop=True)
                stateE[k] = (g, S, aexp)

            def prepL2(k):
                g, S, aexp = stateE.pop(k)
                logits = mp.tile([P, NSUB, H], F32, tag="logits")
                nc.vector.tensor_tensor(out=logits[:], in0=g[:, :, 256:264],
                                        in1=aexp[:], op=mybir.AluOpType.add)
                e1 = mp.tile([P, NSUB, H], F32, tag="e1")
                nc.scalar.activation(e1[:], logits[:], mybir.ActivationFunctionType.Exp)
                e2 = mp.tile([P, NSUB, H], F32, tag="e2")
                nc.scalar.activation(e2[:], logits[:], mybir.ActivationFunctionType.Exp,
                                     scale=0.2)
                msg = mp.tile([P, NSUB, 264], BF, tag="msg")
                wt = mp.tile([P, NSUB, H], BF, tag="wt")
                nc.vector.tensor_tensor(out=wt[:], in0=e1[:], in1=e2[:],
                                        op=mybir.AluOpType.max)
                nc.scalar.copy(msg[:, :, 256:264], wt[:])
                nc.vector.tensor_tensor(
                    out=msg[:, :, 0:256].rearrange("p t (c h) -> p t c h", c=C),
                    in0=g[:, :, 0:256].rearrange("p t (c h) -> p t c h", c=C),
                    in1=wt[:, :, None, :].to_broadcast([P, NSUB, C, H]),
                    op=mybir.AluOpType.mult)
                state[k] = (S, msg)

            def agg2(k):
                b, half = k // 2, k % 2
                S, msg = state.pop(k)
                if half == 0:
                    pblks[b] = ps.tile([P, 264], F32, space="PSUM", tag="accum", name="pblk")
                pblk = pblks[b]
                for t in range(NSUB):
                    nc.tensor.matmul(out=pblk[:], lhsT=S[:, t, :],
                                     rhs=msg[:, t, :],
                                     start=(half == 0 and t == 0),
                                     stop=(half == 1 and t == NSUB - 1))

            def tail1(b):
                pb = pblks.pop(b)
                srec = tailp.tile([P, H], F32, tag="srec")
                nc.vector.tensor_scalar(
                    out=srec[:], in0=pb[:, 256:264], scalar1=1e-16, scalar2=None,
                    op0=mybir.AluOpType.add)
                rec = tailp.tile([P, H], F32, tag="rec")
                nc.vector.reciprocal(rec[:], srec[:])
                out1 = tailp.tile([P, HD], F32, tag="out1")
                nc.vector.tensor_tensor(
                    out=out1[:].rearrange("p (c h) -> p c h", c=C),
                    in0=pb[:, 0:256].rearrange("p (c h) -> p c h", c=C),
                    in1=rec[:, None, :].to_broadcast([P, C, H]),
                    op=mybir.AluOpType.mult)
                v = tailp.tile([P, HD], F32, tag="v")
                nc.vector.tensor_tensor(out=v[:], in0=out1[:], in1=b1bc[:],
                                        op=mybir.AluOpType.add)
                ev = tailp.tile([P, HD], F32, tag="ev")
                nc.scalar.activation(ev[:], v[:], mybir.ActivationFunctionType.Exp)
                em = tailp.tile([P, HD], F32, tag="em")
                nc.vector.tensor_scalar(out=em[:], in0=ev[:], scalar1=1.0, scalar2=0.0,
                                        op0=mybir.AluOpType.subtract,
                                        op1=mybir.AluOpType.min)
                pp = tailp.tile([P, HD], F32, tag="pp")
                nc.vector.tensor_scalar(out=pp[:], in0=v[:], scalar1=0.0, scalar2=None,
                                        op0=mybir.AluOpType.max)
                elu = tailp.tile([P, HD], BF, tag="elu")
                nc.vector.tensor_tensor(out=elu[:], in0=em[:], in1=pp[:],
                                        op=mybir.AluOpType.add)
                eT_ps = ps.tile([P, 2, P], BF, space="PSUM", tag="tps")
                for kk in range(2):
                    nc.tensor.transpose(eT_ps[:, kk, :], elu[:, kk * P:(kk + 1) * P],
                                        ident[:])
                eT_sb = tailp.tile([P, 2, P], BF, tag="eT")
                nc.scalar.copy(eT_sb[:], eT_ps[:])
                ph2 = ps.tile([P, 4], F32, space="PSUM", tag="aexp", bufs=3)
                for kk in range(2):
                    nc.tensor.matmul(out=ph2[:], lhsT=eT_sb[:, kk, :],
                                     rhs=w2e_sb[:, kk, :],
                                     start=(kk == 0), stop=(kk == 1))
                t2row = tailp.tile([P, 4], BF, tag="t2row")
                nc.vector.tensor_copy(t2row[:], ph2[:, 0:4])
                nc.vector.tensor_copy(adst2_sb[:, b:b + 1], ph2[:, 3:4])
                kag = b // CBLK
                rb = (b % CBLK) * P
                nc.sync.dma_start(out=t2locs[kag][rb:rb + P, 0:4], in_=t2row[:])

            for k in range(NB2 + 2):
                if k >= 2:
                    agg2(k - 2)
                    if k % 2 == 1:
                        b = (k - 2) // 2
                        tail1(b)
                        if (b + 1) % CBLK == 0:
                            kag = b // CBLK
                            nc.gpsimd.collective_compute(
                                "AllGather", mybir.AluOpType.bypass,
                                replica_groups=[list(range(NCORES))],
                                ins=[t2locs[kag][:]],
                                outs=[table2[kag * NCORES * CROWS:
                                             (kag + 1) * NCORES * CROWS, :]])
                if k < NB2:
                    prepE2(k)
                if 1 <= k <= NB2:
                    prepL2(k - 1)

            # =========== P3: layer-2 message passing (pipelined) ===========
            def prepE3(k):
                b = k // 2
                half = k % 2
                g2 = g2p.tile([P, NSUB, ELEM2], BF, tag="g2")
                if NEG:
                    creg = nc.gpsimd.alloc_register()
                    nc.gpsimd.reg_load(creg, cnt2_sb[:, k:k + 1])
                    nreg = creg
                else:
                    nreg = NEH
                nc.gpsimd.dma_gather(
                    out_ap=g2[:],
                    in_ap=(table2[0:HALF, :] if half == 0 else table2[HALF:NPAD, :]),
                    idxs_ap=idx2_sb[:, k * IDXW:(k + 1) * IDXW],
                    num_idxs=NEH, num_idxs_reg=nreg, elem_size=ELEM2,
                    single_packet=SP3, queue_num=k % NSWQ)
                S = sp.tile([P, NSUB, P], BF, tag="S")
                nc.vector.tensor_tensor(
                    out=S[:], in0=iota_rep[:],
                    in1=dst2_sb[:, k * NSUB:(k + 1) * NSUB][:, :, None]
                        .to_broadcast([P, NSUB, P]),
                    op=mybir.AluOpType.is_equal)
                ST = sp.tile([P, NSUB, P], BF, tag="ST", bufs=3)
                for grp in range(2):
                    tps = ps.tile([P, 5, P], BF, space="PSUM", tag="tps")
                    for tt in range(5):
                        t = grp * 5 + tt
                        nc.tensor.transpose(tps[:, tt, :], S[:, t, :], ident[:])
                    nc.scalar.copy(ST[:, grp * 5:(grp + 1) * 5, :], tps[:])
                a2e = ps.tile([P, NSUB], F32, space="PSUM", tag="aexp", bufs=3)
                for t in range(NSUB):
                    nc.tensor.matmul(out=a2e[:, t:t + 1], lhsT=ST[:, t, :],
                                     rhs=adst2_sb[:, b:b + 1], start=True, stop=True)
                stateE[k] = (g2, S, a2e)

            def prepL3(k):
                g2, S, a2e = stateE.pop(k)
                lg2 = mp.tile([P, NSUB], F32, tag="lg2")
                nc.vector.tensor_tensor(
                    out=lg2[:], in0=g2[:, :, 2:3].rearrange("p t x -> p (t x)"),
                    in1=a2e[:], op=mybir.AluOpType.add)
                f1 = mp.tile([P, NSUB], F32, tag="f1")
                nc.scalar.activation(f1[:], lg2[:], mybir.ActivationFunctionType.Exp)
                f2 = mp.tile([P, NSUB], F32, tag="f2")
                nc.scalar.activation(f2[:], lg2[:], mybir.ActivationFunctionType.Exp,
                                     scale=0.2)
                msg2 = mp.tile([P, NSUB, 3], BF, tag="msg2")
                w2tv = msg2[:, :, 2:3].rearrange("p t x -> p (t x)")
                nc.vector.tensor_tensor(out=w2tv, in0=f1[:], in1=f2[:],
                                        op=mybir.AluOpType.max)
                nc.vector.tensor_tensor(
                    out=msg2[:, :, 0:2], in0=g2[:, :, 0:2],
                    in1=msg2[:, :, 2:3].to_broadcast([P, NSUB, 2]),
                    op=mybir.AluOpType.mult)
                state[k] = (S, msg2)

            def agg3(k):
                b, half = k // 2, k % 2
                S, msg2 = state.pop(k)
                if half == 0:
                    pblks[b] = ps.tile([P, 3], F32, space="PSUM", tag="accum", name="p2s")
                p2s = pblks[b]
                for t in range(NSUB):
                    nc.tensor.matmul(out=p2s[:], lhsT=S[:, t, :],
                                     rhs=msg2[:, t, :],
                                     start=(half == 0 and t == 0),
                                     stop=(half == 1 and t == NSUB - 1))

            def tail3(b):
                p2s = pblks.pop(b)
                s2r = tailp.tile([P, 1], F32, tag="s2r")
                nc.vector.tensor_scalar(out=s2r[:], in0=p2s[:, 2:3], scalar1=1e-16,
                                        scalar2=None, op0=mybir.AluOpType.add)
                rec2 = tailp.tile([P, 1], F32, tag="rec2")
                nc.vector.reciprocal(rec2[:], s2r[:])
                o2 = tailp.tile([P, 2], F32, tag="o2")
                nc.vector.tensor_tensor(out=o2[:], in0=p2s[:, 0:2],
                                        in1=rec2[:].to_broadcast([P, 2]),
                                        op=mybir.AluOpType.mult)
                nc.vector.tensor_tensor(out=outstage[:, b, :], in0=o2[:], in1=b2bc[:],
                                        op=mybir.AluOpType.add)

            for k in range(NB2 + 2):
                if k >= 2:
                    agg3(k - 2)
                    if k % 2 == 1:
                        tail3((k - 2) // 2)
                if k < NB2:
                    prepE3(k)
                if 1 <= k <= NB2:
                    prepL3(k - 1)

            nc.sync.dma_start(
                out=out_d[:].rearrange("(b p) c -> p b c", p=P), in_=outstage[:])

    nc.compile()
    return nc


def host_prep(inputs, cfg):
    """Build per-core input maps from full inputs."""
    N, NPAD, PER, NBLK, HALF, NSUB, NCORES, NAG = (
        cfg["N"], cfg["NPAD"], cfg["PER"], cfg["NBLK"], cfg["HALF"],
        cfg["NSUB"], cfg["NCORES"], cfg["NAG"])
    NEH = NSUB * P
    IDXW = NEH // 16
    CBLK = NBLK // NAG

    x = np.asarray(inputs["x"], dtype=np.float32)
    ei = np.asarray(inputs["edge_index"], dtype=np.int64)
    W1 = np.asarray(inputs["W1"], dtype=np.float64)
    a1s = np.asarray(inputs["a1_src"], dtype=np.float64)
    a1d = np.asarray(inputs["a1_dst"], dtype=np.float64)
    b1 = np.asarray(inputs["b1"], dtype=np.float32)
    W2 = np.asarray(inputs["W2"], dtype=np.float64)
    a2s = np.asarray(inputs["a2_src"], dtype=np.float64)
    a2d = np.asarray(inputs["a2_dst"], dtype=np.float64)
    b2 = np.asarray(inputs["b2"], dtype=np.float32)

    xT = np.zeros((FIN, NPAD), dtype=np.float32)
    xT[:, :N] = x.T
    xT = xT.astype(NP_BF)

    A1s = np.zeros((HD, H))
    A1d = np.zeros((HD, H))
    for hd in range(H):
        A1s[hd * C:(hd + 1) * C, hd] = a1s[hd]
        A1d[hd * C:(hd + 1) * C, hd] = a1d[hd]
    W1ch = W1.reshape(FIN, H, C).transpose(0, 2, 1).reshape(FIN, HD)
    w1e = np.concatenate([W1ch, W1 @ A1s, W1 @ A1d], axis=1).astype(NP_BF)  # [128,272]

    w2cols = np.concatenate([W2, W2 @ a2s[0][:, None], W2 @ a2d[0][:, None]],
                            axis=1)  # [HD, 4]
    w2cols = w2cols.reshape(H, C, 4).transpose(1, 0, 2).reshape(HD, 4)
    w2e = w2cols.reshape(2, P, 4).transpose(1, 0, 2).reshape(P, 8).astype(NP_BF)

    loops = np.arange(N, dtype=np.int64)
    src = np.concatenate([ei[0], loops])
    dst = np.concatenate([ei[1], loops])

    # chunk-major table2 row index for each node
    nodes = np.arange(NPAD, dtype=np.int64)
    n_c, n_r = nodes // PER, nodes % PER
    n_b, n_j = n_r >> 7, n_r & 127
    n_k = n_b // CBLK
    row2_of = (n_k * (NCORES * CBLK * P) + n_c * (CBLK * P)
               + (n_b - n_k * CBLK) * P + n_j)

    in_maps = []
    for c in range(NCORES):
        lo_n, hi_n = c * PER, (c + 1) * PER
        m = (dst >= lo_n) & (dst < hi_n)
        s_c = src[m]
        d_c = dst[m] - lo_n
        blk = d_c >> 7
        dloc = d_c & 127

        packs = []
        for srow in (s_c, row2_of[s_c]):
            halfsel = (srow >= HALF).astype(np.int64)
            key = blk * 2 + halfsel
            order = np.argsort(key, kind="stable")
            key_s = key[order]
            cnt = np.bincount(key_s, minlength=NBLK * 2)
            assert cnt.max() <= NEH, f"bucket overflow: {cnt.max()} > {NEH}"
            starts = np.zeros(NBLK * 2, dtype=np.int64)
            starts[1:] = np.cumsum(cnt)[:-1]
            pos = np.arange(len(key_s)) - starts[key_s]
            slot = key_s * NEH + pos
            pad_idx = -1 if cfg.get("NEG", 1) else 0
            idxflat = np.full(NBLK * 2 * NEH, pad_idx, dtype=np.int16)
            dstflat = np.full(NBLK * 2 * NEH, -1.0, dtype=np.float32)
            sv = srow[order] - halfsel[order] * HALF
            for eb in np.nonzero(cnt == 0)[0]:
                idxflat[eb * NEH] = 0
            idxflat[slot] = sv.astype(np.int16)
            dstflat[slot] = dloc[order].astype(np.float32)
            idxw16 = (idxflat.reshape(NBLK * 2, NSUB * 8, 16)
                      .transpose(2, 0, 1).reshape(16, -1))
            idxw = np.tile(idxw16, (8, 1))  # replicated across the 8 Q7 cores
            dstw = (dstflat.reshape(NBLK * 2, NSUB, P).transpose(2, 0, 1)
                    .reshape(P, NBLK * 2 * NSUB)).astype(NP_BF)
            cc = (np.maximum(cnt, 1) if cfg.get("NEG", 1)
                  else np.full_like(cnt, NEH))
            packs.append((idxw, np.ascontiguousarray(dstw),
                          cc.reshape(1, -1).astype(np.int32)))

        in_maps.append({
            "xT": xT, "w1e": w1e, "w2e": w2e,
            "b1r": b1.reshape(H, C).T.reshape(1, HD).astype(np.float32),
            "b2r": b2.reshape(1, 2).astype(np.float32),
            "idx16": packs[0][0], "dstf": packs[0][1], "cnts": packs[0][2],
            "idx16b": packs[1][0], "dstfb": packs[1][1], "cnts2": packs[1][2],
        })
    return in_maps


_NC_CACHE = {}


def _get_nc():
    if "nc" not in _NC_CACHE:
        _NC_CACHE["nc"] = build_nc(FULL_CFG)
    return _NC_CACHE["nc"]


def kernel(**inputs):
    from concourse.bass_utils import run_bass_kernel_spmd

    nc = _get_nc()
    in_maps = host_prep(inputs, FULL_CFG)
    res = run_bass_kernel_spmd(nc, in_maps, core_ids=list(range(FULL_CFG["NCORES"])))
    out = np.concatenate([r["out"] for r in res.results])[:FULL_CFG["N"]]
    return np.ascontiguousarray(out.astype(np.float32))
